# revision 5
# baseline (speedup 1.0000x reference)
"""Trainium2 Bass kernel for the CIFAR10 Monarch MLP (7 monarch layers + log_softmax).

Strategy
--------
Pure data parallel over 8 NeuronCores: each core takes a 1024-row batch shard;
weights are replicated. Activations are feature-major in SBUF
([feature-tile partitions, batch free dim]), fully SBUF-resident; only x,
weights and final log-probs cross HBM.

Performance structure (vs the v1 baseline):
- Layers 1-2 run in fp8(e4m3) with DoubleRow matmuls: each PE instruction
  contracts 256 rows (two 128-row tiles packed in the stationary/moving
  operands), 2x the bf16 rate. Partial 128-row tiles are zero-padded
  (host-side for weights, gpsimd memset for activations) so pairs can always
  contract over a full 128 partitions.
- Layers 3+ run in fp16 (same PE rate as bf16, 8x finer mantissa, which
  leaves the error budget to the fp8 layers).
- Layers 4-6 are folded into single dense GEMMs (effective W1*P*W2 built on
  the host): the block-diag structure is too fine for 128-wide tiles there,
  so dense has fewer matmuls and no mid eviction on the critical path.
- Layer 7 + log_softmax: logits are produced *batch-major* by using the
  activations as the stationary operand (out[batch,12] = h6_chunk.T @ W7),
  which kills the transposes; softmax skips max-subtraction (|logit| < 1)
  and runs two-pass (all Exp accumulations, then one Ln) so the scalar
  engine loads each activation table exactly once.
- Head: x and layer-1 weights are DMAed in interleaved (k-block, col-half)
  chunks ordered so the first matmul can start ~10us in, instead of waiting
  ~38us for everything.

Config via KERNEL_CFG: "A" = fp8 L1+L2 (default), "B" = fp8 L1 only,
"C" = all fp16.
"""

import os as _os

import numpy as np

import concourse.bacc as bacc_mod
import concourse.mybir as mybir
import concourse.tile as tile
from concourse.bass_utils import run_bass_kernel_spmd

# ----------------------------------------------------------------- problem dims
BATCH = 8192
IN_FEATURES = 3072
NCORES = 8
BPC = BATCH // NCORES          # 1024 batch rows per core
NOUT = 10

SHAPES = [((4, 750, 768), (4, 750, 750)),
          ((4, 500, 750), (4, 500, 500)),
          ((4, 250, 500), (4, 250, 250)),
          ((4, 125, 250), (4, 125, 125)),
          ((4, 50, 125), (4, 50, 50)),
          ((4, 25, 50), (4, 25, 25)),
          ((4, 3, 25), (4, 3, 3))]
NLAYERS = 7
NMONARCH = 3                   # layers emitted as 2 block-sparse GEMMs
CS = 2                         # batch column chunks (512 wide)
CW = BPC // CS

F32 = mybir.dt.float32
F16 = mybir.dt.float16
FP8 = mybir.dt.float8e4

CFG = _os.environ.get("KERNEL_CFG", "A")
NFP8 = {"A": 2, "B": 1, "C": 0}[CFG]
LDT = [FP8 if i < NFP8 else F16 for i in range(NLAYERS)]


# ------------------------------------------------------------------ layouts
class Layout:
    """Placement of 4 feature blocks of size Sb into 128-partition tiles."""

    @classmethod
    def from_positions(cls, Sb, ntiles, feat_tile, feat_row):
        self = object.__new__(cls)
        self.Sb = Sb
        self.ntiles = ntiles
        self.feat_tile = feat_tile
        self.feat_row = feat_row
        self._finish()
        return self

    def _finish(self):
        self.valid = np.zeros(self.ntiles, np.int64)
        for k in range(4):
            for t, r in zip(self.feat_tile[k], self.feat_row[k]):
                self.valid[t] = max(self.valid[t], r + 1)
        self.grow = [self.feat_tile[k] * 128 + self.feat_row[k]
                     for k in range(4)]
        self.tiles_of_block = [sorted(set(self.feat_tile[k].tolist()))
                               for k in range(4)]


def simple_layout(Sb):
    if Sb >= 128:
        cpb = (Sb + 127) // 128
        ft, fr = [], []
        for k in range(4):
            i = np.arange(Sb)
            ft.append(k * cpb + i // 128)
            fr.append(i % 128)
        return Layout.from_positions(Sb, 4 * cpb, ft, fr)
    stride = ((Sb + 31) // 32) * 32
    bpt = max(1, 128 // stride)
    ntiles = (4 + bpt - 1) // bpt
    ft, fr = [], []
    for k in range(4):
        i = np.arange(Sb)
        ft.append(np.full(Sb, k // bpt, np.int64))
        fr.append((k % bpt) * stride + i)
    return Layout.from_positions(Sb, ntiles, ft, fr)


def grouped_mid_layout(R, Q):
    """Mid layout with features regrouped by input block k (R >= 125)."""
    cpb = max(1, (R + 127) // 128)
    block_rows = cpb * 128
    Gp = block_rows // 4
    ft, fr = [], []
    for l in range(4):
        rs = np.arange(R)
        ks = (4 * rs + l) // Q
        pos = np.empty(R, np.int64)
        for k in range(4):
            idx = rs[ks == k]
            assert len(idx) <= Gp
            pos[idx] = k * Gp + np.arange(len(idx))
        ft.append(l * cpb + pos // 128)
        fr.append(pos % 128)
    return Layout.from_positions(R, 4 * cpb, ft, fr)


def dense_mats(w1, w2, lin, lmid, lout, Q):
    """Dense effective W1full [in_ext, mid_ext], W2full [mid_ext, out_ext]."""
    _, S, R = w2.shape
    W1full = np.zeros((lin.ntiles * 128, lmid.ntiles * 128), np.float32)
    W2full = np.zeros((lmid.ntiles * 128, lout.ntiles * 128), np.float32)
    for l in range(4):
        js = 4 * np.arange(R) + l
        ks, qs = js // Q, js % Q
        mcols = lmid.grow[l]
        for k in range(4):
            sel = np.where(ks == k)[0]
            if len(sel) == 0:
                continue
            W1full[np.ix_(lin.grow[k], mcols[sel])] = \
                np.ascontiguousarray(w1[k, qs[sel], :].T)
        W2full[np.ix_(lmid.grow[l], lout.grow[l])] = \
            np.ascontiguousarray(w2[l].T)
    return W1full, W2full


class LayerPlan:
    """Monarch layer as two block-sparse GEMMs (layers 1..NMONARCH)."""

    def __init__(self, li, w1_shape, w2_shape, in_layout):
        _, Q, P = w1_shape
        _, S, R = w2_shape
        self.li, self.P, self.Q, self.R, self.S = li, P, Q, R, S
        self.lin = in_layout
        self.lmid = grouped_mid_layout(R, Q)
        self.lout = simple_layout(S)
        self.fused = False
        self.dr = LDT[li] == FP8
        self._build()

    def _build(self):
        Q, R, S = self.Q, self.R, self.S
        ks_of = [(4 * np.arange(R) + l) // Q for l in range(4)]

        need1 = {}
        for l in range(4):
            for r in range(R):
                mt = int(self.lmid.feat_tile[l][r])
                k = int(ks_of[l][r])
                need1.setdefault(mt, set()).update(self.lin.tiles_of_block[k])
        self.g1_chains = {mt: sorted(its) for mt, its in need1.items()}
        self.w1_blocks = [(mt, it) for mt in sorted(need1)
                          for it in self.g1_chains[mt]]
        self.w1_block_of = {p: i for i, p in enumerate(self.w1_blocks)}

        need2 = {}
        for l in range(4):
            for s in range(S):
                ot = int(self.lout.feat_tile[l][s])
                need2.setdefault(ot, set()).update(self.lmid.tiles_of_block[l])
        self.g2_chains = {ot: sorted(mts) for ot, mts in need2.items()}
        self.w2_blocks = [(ot, mt) for ot in sorted(need2)
                          for mt in self.g2_chains[ot]]
        self.w2_block_of = {p: i for i, p in enumerate(self.w2_blocks)}

        self.mid_tiles_of_l = [self.lmid.tiles_of_block[l] for l in range(4)]
        self.out_tiles_of_l = [self.lout.tiles_of_block[l] for l in range(4)]
        if self.dr:
            for mt, its in self.g1_chains.items():
                # pairs must not straddle input blocks (rhs tiles must be
                # adjacent in SBUF); every k-segment here is even-length
                for k in sorted({t // len(self.lin.tiles_of_block[0])
                                 for t in its}):
                    seg = [t for t in its if t in self.lin.tiles_of_block[k]]
                    assert len(seg) % 2 == 0
            for ot, mts in self.g2_chains.items():
                assert len(mts) % 2 == 0

        # w1 block ranges per mid-block l (for per-l weight tiles)
        self.w1_range_of_l = []
        for l in range(4):
            mts = self.mid_tiles_of_l[l]
            idxs = [i for i, (mt, _) in enumerate(self.w1_blocks) if mt in mts]
            assert idxs == list(range(idxs[0], idxs[0] + len(idxs)))
            self.w1_range_of_l.append((idxs[0], len(idxs)))
        self.w2_range_of_l = []
        for l in range(4):
            ots = self.out_tiles_of_l[l]
            idxs = [i for i, (ot, _) in enumerate(self.w2_blocks) if ot in ots]
            assert idxs == list(range(idxs[0], idxs[0] + len(idxs)))
            self.w2_range_of_l.append((idxs[0], len(idxs)))

    def build_weights(self, w1, w2):
        W1full, W2full = dense_mats(w1, w2, self.lin, self.lmid, self.lout,
                                    self.Q)
        W1m = np.zeros((128, 128 * len(self.w1_blocks)), np.float32)
        for i, (mt, it) in enumerate(self.w1_blocks):
            W1m[:, i * 128:(i + 1) * 128] = \
                W1full[it * 128:(it + 1) * 128, mt * 128:(mt + 1) * 128]
        W2m = np.zeros((128, 128 * len(self.w2_blocks)), np.float32)
        for i, (ot, mt) in enumerate(self.w2_blocks):
            W2m[:, i * 128:(i + 1) * 128] = \
                W2full[mt * 128:(mt + 1) * 128, ot * 128:(ot + 1) * 128]
        return W1m, W2m


class FusedPlan:
    """Layers 4-6: one dense GEMM over the effective layer matrix."""

    def __init__(self, li, w1_shape, w2_shape, in_layout):
        _, Q, P = w1_shape
        _, S, R = w2_shape
        self.li, self.Q, self.R, self.S = li, Q, R, S
        self.lin = in_layout
        self.lmid = simple_layout(R)       # host-only intermediate
        self.lout = simple_layout(S)
        self.fused = True
        self.dr = False
        its = list(range(self.lin.ntiles))
        self.chains = {ot: its for ot in range(self.lout.ntiles)}
        self.blocks = [(ot, it) for ot in range(self.lout.ntiles)
                       for it in its]
        self.block_of = {p: i for i, p in enumerate(self.blocks)}

    def build_weights(self, w1, w2):
        W1full, W2full = dense_mats(w1, w2, self.lin, self.lmid, self.lout,
                                    self.Q)
        E = W1full @ W2full
        Em = np.zeros((128, 128 * len(self.blocks)), np.float32)
        for i, (ot, it) in enumerate(self.blocks):
            Em[:, i * 128:(i + 1) * 128] = \
                E[it * 128:(it + 1) * 128, ot * 128:(ot + 1) * 128]
        return Em


class FinalPlan:
    """Layer 7: dense effective [in_ext, 12], consumed batch-major."""

    def __init__(self, li, w1_shape, w2_shape, in_layout):
        _, Q, P = w1_shape
        _, S, R = w2_shape
        self.li, self.Q, self.R, self.S = li, Q, R, S
        self.lin = in_layout
        assert self.lin.ntiles == 1
        self.lmid = simple_layout(R)
        self.lout = simple_layout(S)
        self.fused = True
        self.dr = False

    def build_weights(self, w1, w2):
        W1full, W2full = dense_mats(w1, w2, self.lin, self.lmid, self.lout,
                                    self.Q)
        E = W1full @ W2full
        cols = [self.lout.grow[l][s] for l in range(4) for s in range(3)]
        return np.ascontiguousarray(E[:, cols[:NOUT]])   # [128, 10]


def build_plans():
    plans = []
    lin = simple_layout(SHAPES[0][0][2])
    for i, (s1, s2) in enumerate(SHAPES):
        if i < NMONARCH:
            pl = LayerPlan(i, s1, s2, lin)
        elif i < NLAYERS - 1:
            pl = FusedPlan(i, s1, s2, lin)
        else:
            pl = FinalPlan(i, s1, s2, lin)
        plans.append(pl)
        lin = pl.lout
    return plans


# ------------------------------------------------------------------ bass program
def build_program(plans):
    nc = bacc_mod.Bacc()

    p0 = plans[0]
    XT_T = p0.lin.ntiles                     # 24 input tiles
    KT = XT_T // 4                           # tiles per input block
    xT = nc.dram_tensor("xT", [XT_T, 128, BPC], LDT[0], kind="ExternalInput")
    w1d, w2d, wfd = {}, {}, {}
    for i, pl in enumerate(plans):
        if not pl.fused:
            w1d[i] = nc.dram_tensor(f"w1c_{i}", [128, 128 * len(pl.w1_blocks)],
                                    LDT[i], kind="ExternalInput")
            w2d[i] = nc.dram_tensor(f"w2c_{i}", [128, 128 * len(pl.w2_blocks)],
                                    LDT[i], kind="ExternalInput")
        elif i < NLAYERS - 1:
            wfd[i] = nc.dram_tensor(f"wc_{i}", [128, 128 * len(pl.blocks)],
                                    F16, kind="ExternalInput")
        else:
            wfd[i] = nc.dram_tensor(f"wc_{i}", [128, NOUT], F16,
                                    kind="ExternalInput")
    y = nc.dram_tensor("y", [BPC, NOUT], F32, kind="ExternalOutput")

    with tile.TileContext(nc) as tc:
        with (
            tc.tile_pool(name="sb", bufs=1) as sb,
            tc.tile_pool(name="ps", bufs=1, space="PSUM") as ps,
        ):
            evict_flip = [0]

            def evict(dst_ap, src_ap, relu):
                e = evict_flip[0] = evict_flip[0] ^ 1
                if relu:
                    if e:
                        nc.vector.tensor_scalar_max(dst_ap, src_ap, 0.0)
                    else:
                        nc.scalar.activation(dst_ap, src_ap,
                                             mybir.ActivationFunctionType.Relu)
                else:
                    if e:
                        nc.vector.tensor_copy(dst_ap, src_ap)
                    else:
                        nc.scalar.copy(dst_ap, src_ap)

            def zero_pads(tile_ap, layout, loc_tiles, cols):
                """memset pad rows of partial tiles (needed under DoubleRow).

                Engines require a 32-aligned partition base, so zero from the
                aligned floor; the eviction that follows overwrites the
                overlap with real data (WAW ordering handles it)."""
                for loc, t in enumerate(loc_tiles):
                    v = int(layout.valid[t])
                    if v < 128:
                        v0 = (v // 32) * 32
                        nc.gpsimd.memset(tile_ap[v0:128, loc, 0:cols], 0)

            # =========================== layer 1 ===========================
            # x chunk tiles, one DMA each: hx[k][cs] = [128, KT, CW]
            hx = [[sb.tile([128, KT, CW], LDT[0], name=f"x_{k}_{cs}",
                           tag=f"hx{k}{cs}") for cs in range(CS)]
                  for k in range(4)]

            def dma_x(k, cs):
                nc.sync.dma_start(
                    out=hx[k][cs][:, :, :],
                    in_=xT[k * KT:(k + 1) * KT, :, cs * CW:(cs + 1) * CW]
                    .rearrange("t p n -> p t n"))

            G1 = 1 if LDT[0] == FP8 else 2    # weight groups (SBUF pressure)
            w1t, w2t = {}, {}

            def dma_w1_l(l):
                b0, nb = p0.w1_range_of_l[l]
                tg = l if G1 == 1 else l % 2
                w1t[l] = sb.tile([128, nb, 128], LDT[0], name=f"w1L0_{l}",
                                 tag=f"w1L0_{tg}")
                nc.sync.dma_start(
                    out=w1t[l][:, :, :],
                    in_=w1d[0][:, b0 * 128:(b0 + nb) * 128]
                    .rearrange("p (n m) -> p n m", m=128))

            def dma_w2_l(l):
                b0, nb = p0.w2_range_of_l[l]
                tg = l if G1 == 1 else l % 2
                w2t[l] = sb.tile([128, nb, 128], LDT[0], name=f"w2L0_{l}",
                                 tag=f"w2L0_{tg}")
                nc.sync.dma_start(
                    out=w2t[l][:, :, :],
                    in_=w2d[0][:, b0 * 128:(b0 + nb) * 128]
                    .rearrange("p (n m) -> p n m", m=128))

            h2 = sb.tile([128, p0.lout.ntiles, BPC], LDT[1], name="h_1",
                         tag="hB")
            if plans[1].dr:
                zero_pads(h2, p0.lout, range(p0.lout.ntiles), BPC)

            def l1_g1_chain(l, mt, mtloc, midl, cs):
                V = int(p0.lmid.valid[mt])
                its = p0.g1_chains[mt]
                b0_l = p0.w1_range_of_l[l][0]
                pm = ps.tile([128, CW], F32, name="pm_0", tag="pmid", bufs=3)
                if p0.dr:
                    j = 0
                    first = True
                    while j < len(its):
                        t0 = its[j]
                        k = t0 // KT
                        b = p0.w1_block_of[(mt, t0)] - b0_l
                        lhsT = w1t[l][:, b:b + 2, 0:V]
                        rhs = hx[k][cs][:, (t0 % KT):(t0 % KT) + 2, :]
                        nc.tensor.matmul(pm[0:V, :], lhsT, rhs,
                                         start=first, stop=(j + 2 >= len(its)),
                                         perf_mode=mybir.MatmulPerfMode.DoubleRow)
                        first = False
                        j += 2
                else:
                    for j, it in enumerate(its):
                        ln = int(p0.lin.valid[it])
                        k = it // KT
                        b = p0.w1_block_of[(mt, it)] - b0_l
                        lhsT = w1t[l][0:ln, b, 0:V]
                        rhs = hx[k][cs][0:ln, it % KT, :]
                        nc.tensor.matmul(pm[0:V, :], lhsT, rhs,
                                         start=(j == 0),
                                         stop=(j == len(its) - 1))
                evict(midl[0:V, mtloc, :], pm[0:V, :], relu=False)

            def l1_g2_chain(l, ot, mid_of, cs):
                V = int(p0.lout.valid[ot])
                mts = p0.g2_chains[ot]
                b0_l = p0.w2_range_of_l[l][0]
                po = ps.tile([128, CW], F32, name="po_0", tag="pout", bufs=3)
                if p0.dr:
                    for j in range(0, len(mts), 2):
                        mt = mts[j]
                        b = p0.w2_block_of[(ot, mt)] - b0_l
                        midl, loc = mid_of[mt]
                        lhsT = w2t[l][:, b:b + 2, 0:V]
                        rhs = midl[:, loc:loc + 2, :]
                        nc.tensor.matmul(po[0:V, :], lhsT, rhs,
                                         start=(j == 0),
                                         stop=(j + 2 >= len(mts)),
                                         perf_mode=mybir.MatmulPerfMode.DoubleRow)
                else:
                    for j, mt in enumerate(mts):
                        ln = int(p0.lmid.valid[mt])
                        b = p0.w2_block_of[(ot, mt)] - b0_l
                        midl, loc = mid_of[mt]
                        lhsT = w2t[l][0:ln, b, 0:V]
                        rhs = midl[0:ln, loc, :]
                        nc.tensor.matmul(po[0:V, :], lhsT, rhs,
                                         start=(j == 0),
                                         stop=(j == len(mts) - 1))
                evict(h2[0:V, ot, cs * CW:(cs + 1) * CW], po[0:V, :],
                      relu=True)

            # head DMA interleave + L1 emission (cs-outer, per-l lookahead)
            for g in range(G1):
                ls = list(range(4)) if G1 == 1 else [2 * g, 2 * g + 1]
                if g == 0:
                    dma_x(0, 0)
                    dma_w1_l(ls[0])
                    dma_x(1, 0)
                    dma_w1_l(ls[1])
                    dma_x(2, 0)
                    dma_x(3, 0)
                    dma_w2_l(ls[0])
                    if G1 == 1:
                        dma_w1_l(2)
                    dma_x(0, 1)
                    dma_x(1, 1)
                    dma_w2_l(ls[1])
                    if G1 == 1:
                        dma_w1_l(3)
                    dma_x(2, 1)
                    dma_x(3, 1)
                    if G1 == 1:
                        dma_w2_l(2)
                        dma_w2_l(3)
                else:
                    dma_w1_l(2)
                    dma_w1_l(3)
                    dma_w2_l(2)
                    dma_w2_l(3)

                for cs in range(CS):
                    mid_of = {}
                    pend = None
                    for l in ls:
                        mts_l = p0.mid_tiles_of_l[l]
                        midl = sb.tile([128, len(mts_l), CW], LDT[0],
                                       name=f"mid_0_{l}_{cs}", tag="midb",
                                       bufs=3)
                        if p0.dr:
                            zero_pads(midl, p0.lmid, mts_l, CW)
                        for loc, mt in enumerate(mts_l):
                            mid_of[mt] = (midl, loc)
                            l1_g1_chain(l, mt, loc, midl, cs)
                        if pend is not None:
                            for ot in p0.out_tiles_of_l[pend]:
                                l1_g2_chain(pend, ot, mid_of, cs)
                        pend = l
                    for ot in p0.out_tiles_of_l[pend]:
                        l1_g2_chain(pend, ot, mid_of, cs)

            # ====================== layers 2..NMONARCH ======================
            hin = h2
            for li in range(1, NMONARCH):
                pl = plans[li]
                w1sb = sb.tile([128, len(pl.w1_blocks), 128], LDT[li],
                               name=f"w1sb_{li}", tag="w1")
                nc.sync.dma_start(
                    out=w1sb[:, :, :],
                    in_=w1d[li][:, :].rearrange("p (n m) -> p n m", m=128))
                w2sb = sb.tile([128, len(pl.w2_blocks), 128], LDT[li],
                               name=f"w2sb_{li}", tag="w2")
                nc.sync.dma_start(
                    out=w2sb[:, :, :],
                    in_=w2d[li][:, :].rearrange("p (n m) -> p n m", m=128))

                hnext = sb.tile([128, pl.lout.ntiles, BPC], LDT[li + 1],
                                name=f"h_{li + 1}",
                                tag="hA" if li % 2 == 1 else "hB")
                if plans[li + 1].dr:
                    zero_pads(hnext, pl.lout, range(pl.lout.ntiles), BPC)

                def g1_tile(mt, mtloc, midl, cs):
                    V = int(pl.lmid.valid[mt])
                    its = pl.g1_chains[mt]
                    c0 = cs * CW
                    pm = ps.tile([128, CW], F32, name=f"pm_{li}", tag="pmid",
                                 bufs=3)
                    if pl.dr:
                        for j in range(0, len(its), 2):
                            it = its[j]
                            b = pl.w1_block_of[(mt, it)]
                            lhsT = w1sb[:, b:b + 2, 0:V]
                            rhs = hin[:, it:it + 2, c0:c0 + CW]
                            nc.tensor.matmul(
                                pm[0:V, :], lhsT, rhs,
                                start=(j == 0), stop=(j + 2 >= len(its)),
                                perf_mode=mybir.MatmulPerfMode.DoubleRow)
                    else:
                        for j, it in enumerate(its):
                            ln = int(pl.lin.valid[it])
                            b = pl.w1_block_of[(mt, it)]
                            lhsT = w1sb[0:ln, b, 0:V]
                            rhs = hin[0:ln, it, c0:c0 + CW]
                            nc.tensor.matmul(pm[0:V, :], lhsT, rhs,
                                             start=(j == 0),
                                             stop=(j == len(its) - 1))
                    evict(midl[0:V, mtloc, c0:c0 + CW], pm[0:V, :],
                          relu=False)

                def g2_tile(ot, mid_of, cs):
                    V = int(pl.lout.valid[ot])
                    mts = pl.g2_chains[ot]
                    c0 = cs * CW
                    po = ps.tile([128, CW], F32, name=f"po_{li}", tag="pout",
                                 bufs=3)
                    if pl.dr:
                        for j in range(0, len(mts), 2):
                            mt = mts[j]
                            b = pl.w2_block_of[(ot, mt)]
                            midl, loc = mid_of[mt]
                            lhsT = w2sb[:, b:b + 2, 0:V]
                            rhs = midl[:, loc:loc + 2, c0:c0 + CW]
                            nc.tensor.matmul(
                                po[0:V, :], lhsT, rhs,
                                start=(j == 0), stop=(j + 2 >= len(mts)),
                                perf_mode=mybir.MatmulPerfMode.DoubleRow)
                    else:
                        for j, mt in enumerate(mts):
                            ln = int(pl.lmid.valid[mt])
                            b = pl.w2_block_of[(ot, mt)]
                            midl, loc = mid_of[mt]
                            lhsT = w2sb[0:ln, b, 0:V]
                            rhs = midl[0:ln, loc, c0:c0 + CW]
                            nc.tensor.matmul(po[0:V, :], lhsT, rhs,
                                             start=(j == 0),
                                             stop=(j == len(mts) - 1))
                    evict(hnext[0:V, ot, c0:c0 + CW], po[0:V, :], relu=True)

                mid_of = {}
                pend = None
                for l in range(4):
                    mts_l = pl.mid_tiles_of_l[l]
                    midl = sb.tile([128, len(mts_l), BPC], LDT[li],
                                   name=f"mid_{li}_{l}", tag="midb2", bufs=2)
                    if pl.dr:
                        zero_pads(midl, pl.lmid, mts_l, BPC)
                    for loc, mt in enumerate(mts_l):
                        mid_of[mt] = (midl, loc)
                        for cs in range(CS):
                            g1_tile(mt, loc, midl, cs)
                    if pend is not None:
                        for ot in pl.out_tiles_of_l[pend]:
                            for cs in range(CS):
                                g2_tile(ot, mid_of, cs)
                    pend = l
                for ot in pl.out_tiles_of_l[pend]:
                    for cs in range(CS):
                        g2_tile(ot, mid_of, cs)
                hin = hnext

            # ======================= fused layers 4-6 =======================
            for li in range(NMONARCH, NLAYERS - 1):
                pl = plans[li]
                wsb = sb.tile([128, len(pl.blocks), 128], F16,
                              name=f"wsb_{li}", tag="w1")
                nc.sync.dma_start(
                    out=wsb[:, :, :],
                    in_=wfd[li][:, :].rearrange("p (n m) -> p n m", m=128))
                hnext = sb.tile([128, pl.lout.ntiles, BPC], F16,
                                name=f"h_{li + 1}",
                                tag="hA" if li % 2 == 1 else "hB")
                for cs in range(CS):
                    c0 = cs * CW
                    for ot in range(pl.lout.ntiles):
                        V = int(pl.lout.valid[ot])
                        its = pl.chains[ot]
                        po = ps.tile([128, CW], F32, name=f"po_{li}",
                                     tag="pout", bufs=3)
                        for j, it in enumerate(its):
                            ln = int(pl.lin.valid[it])
                            b = pl.block_of[(ot, it)]
                            nc.tensor.matmul(po[0:V, :], wsb[0:ln, b, 0:V],
                                             hin[0:ln, it, c0:c0 + CW],
                                             start=(j == 0),
                                             stop=(j == len(its) - 1))
                        evict(hnext[0:V, ot, c0:c0 + CW], po[0:V, :],
                              relu=True)
                hin = hnext

            # ============== layer 7 (batch-major) + log_softmax ==============
            pf_pl = plans[NLAYERS - 1]
            VF = int(pf_pl.lin.valid[0])
            w7sb = sb.tile([128, NOUT], F16, name="w7sb", tag="w7")
            nc.sync.dma_start(out=w7sb[:, :], in_=wfd[NLAYERS - 1][:, :])

            nchunk = BPC // 128
            esum = sb.tile([128, nchunk], F32, name="esum", tag="esum")
            tlog = sb.tile([128, nchunk, NOUT], F32, name="tlog", tag="tlog")
            osb = sb.tile([128, nchunk, NOUT], F32, name="osb", tag="osb")
            lse = sb.tile([128, nchunk], F32, name="lse", tag="lse")
            for bc in range(nchunk):
                pf = ps.tile([128, NOUT], F32, name="pfin", tag="pfin",
                             bufs=2)
                nc.tensor.matmul(pf[:, :],
                                 hin[0:VF, 0, bc * 128:(bc + 1) * 128],
                                 w7sb[0:VF, :], start=True, stop=True)
                esb = sb.tile([128, NOUT], F32, name="esb", tag="esb",
                              bufs=2)
                nc.scalar.activation(esb, pf,
                                     mybir.ActivationFunctionType.Exp,
                                     accum_out=esum[:, bc:bc + 1])
                nc.vector.tensor_copy(tlog[:, bc, :], pf)
            nc.scalar.activation(lse, esum, mybir.ActivationFunctionType.Ln)
            for bc in range(nchunk):
                nc.vector.tensor_scalar_sub(osb[:, bc, :], tlog[:, bc, :],
                                            lse[:, bc:bc + 1])
            nc.sync.dma_start(
                out=y[:, :].rearrange("(c p) f -> p c f", p=128),
                in_=osb[:, :, :])
    nc.finalize()
    return nc


# --------------------------------------------------- numpy model of the schedule
def numpy_forward(plans, weights, xT):
    """Mirror the device schedule (incl. quantization) for validation."""
    import ml_dtypes

    def npdt(dt):
        return mybir.dt.np(dt)

    B = xT.shape[1]
    h = np.zeros((plans[0].lin.ntiles * 128, B), np.float32)
    h[:xT.shape[0]] = xT.astype(npdt(LDT[0])).astype(np.float32)
    for pl in plans[:NLAYERS - 1]:
        li = pl.li
        if not pl.fused:
            W1m, W2m = weights[li]
            W1m = W1m.astype(np.float32)
            W2m = W2m.astype(np.float32)
            mid = np.zeros((pl.lmid.ntiles * 128, B), np.float32)
            for mt, its in pl.g1_chains.items():
                V = int(pl.lmid.valid[mt])
                acc = np.zeros((V, B), np.float32)
                for it in its:
                    ln = int(pl.lin.valid[it])
                    b = pl.w1_block_of[(mt, it)]
                    acc += W1m[0:ln, b * 128:b * 128 + V].T @ \
                        h[it * 128: it * 128 + ln]
                mid[mt * 128: mt * 128 + V] = acc
            mid = mid.astype(npdt(LDT[li])).astype(np.float32)
            out = np.zeros((pl.lout.ntiles * 128, B), np.float32)
            for ot, mts in pl.g2_chains.items():
                V = int(pl.lout.valid[ot])
                acc = np.zeros((V, B), np.float32)
                for mt in mts:
                    ln = int(pl.lmid.valid[mt])
                    b = pl.w2_block_of[(ot, mt)]
                    acc += W2m[0:ln, b * 128:b * 128 + V].T @ \
                        mid[mt * 128: mt * 128 + ln]
                out[ot * 128: ot * 128 + V] = acc
        else:
            Em = weights[li].astype(np.float32)
            out = np.zeros((pl.lout.ntiles * 128, B), np.float32)
            for ot, its in pl.chains.items():
                V = int(pl.lout.valid[ot])
                acc = np.zeros((V, B), np.float32)
                for it in its:
                    ln = int(pl.lin.valid[it])
                    b = pl.block_of[(ot, it)]
                    acc += Em[0:ln, b * 128:b * 128 + V].T @ \
                        h[it * 128: it * 128 + ln]
                out[ot * 128: ot * 128 + V] = acc
        out = np.maximum(out, 0.0)
        h = out.astype(npdt(LDT[li + 1])).astype(np.float32)
    E7 = weights[NLAYERS - 1].astype(np.float32)
    VF = int(plans[NLAYERS - 1].lin.valid[0])
    logits = (E7[0:VF, :].T @ h[0:VF]).T
    S = np.exp(logits).sum(axis=1, keepdims=True)
    return logits - np.log(S)


# ------------------------------------------------------------------ entry point
def _prep_inputs(inputs, plans):
    x = np.ascontiguousarray(np.asarray(inputs["x"], dtype=np.float32))
    shared = {}
    for i, pl in enumerate(plans):
        w1 = np.asarray(inputs[f"w1_{i + 1}"], dtype=np.float32)
        w2 = np.asarray(inputs[f"w2_{i + 1}"], dtype=np.float32)
        if not pl.fused:
            np_dt = mybir.dt.np(LDT[i])
            W1m, W2m = pl.build_weights(w1, w2)
            shared[f"w1c_{i}"] = np.ascontiguousarray(W1m.astype(np_dt))
            shared[f"w2c_{i}"] = np.ascontiguousarray(W2m.astype(np_dt))
        else:
            Em = pl.build_weights(w1, w2)
            shared[f"wc_{i}"] = np.ascontiguousarray(
                Em.astype(mybir.dt.np(F16)))
    np_x = mybir.dt.np(LDT[0])
    in_maps = []
    for c in range(NCORES):
        m = dict(shared)
        xc = x[c * BPC:(c + 1) * BPC].T.astype(np_x)
        m["xT"] = np.ascontiguousarray(
            xc.reshape(plans[0].lin.ntiles, 128, BPC))
        in_maps.append(m)
    return in_maps


def _run(inputs, trace=False, **spmd_kwargs):
    plans = build_plans()
    in_maps = _prep_inputs(inputs, plans)
    nc = build_program(plans)
    res = run_bass_kernel_spmd(nc, in_maps, core_ids=list(range(NCORES)),
                               trace=trace, **spmd_kwargs)
    out = np.concatenate([r["y"] for r in res.results], axis=0)
    return out.astype(np.float32), res


def kernel(**inputs):
    out, _ = _run(inputs, trace=False)
    return out


# revision 6
# speedup vs baseline: 1.0430x; 1.0430x over previous
"""Trainium2 Bass kernel for the CIFAR10 Monarch MLP (7 monarch layers + log_softmax).

Strategy
--------
Pure data parallel over 8 NeuronCores: each core takes a 1024-row batch shard;
weights are replicated. Activations are feature-major in SBUF
([feature-tile partitions, batch free dim]), fully SBUF-resident; only x,
weights and final log-probs cross HBM.

Performance structure (vs the v1 baseline):
- Layers 1-2 run in fp8(e4m3) with DoubleRow matmuls: each PE instruction
  contracts 256 rows (two 128-row tiles packed in the stationary/moving
  operands), 2x the bf16 rate. Partial 128-row tiles are zero-padded
  (host-side for weights, gpsimd memset for activations) so pairs can always
  contract over a full 128 partitions.
- Layers 3+ run in fp16 (same PE rate as bf16, 8x finer mantissa, which
  leaves the error budget to the fp8 layers).
- Layers 4-6 are folded into single dense GEMMs (effective W1*P*W2 built on
  the host): the block-diag structure is too fine for 128-wide tiles there,
  so dense has fewer matmuls and no mid eviction on the critical path.
- Layer 7 + log_softmax: logits are produced *batch-major* by using the
  activations as the stationary operand (out[batch,12] = h6_chunk.T @ W7),
  which kills the transposes; softmax skips max-subtraction (|logit| < 1)
  and runs two-pass (all Exp accumulations, then one Ln) so the scalar
  engine loads each activation table exactly once.
- Head: x and layer-1 weights are DMAed in interleaved (k-block, col-half)
  chunks ordered so the first matmul can start ~10us in, instead of waiting
  ~38us for everything.

Config via KERNEL_CFG: "A" = fp8 L1+L2 (default), "B" = fp8 L1 only,
"C" = all fp16.
"""

import os as _os

import numpy as np

import concourse.bacc as bacc_mod
import concourse.mybir as mybir
import concourse.tile as tile
from concourse.bass_utils import run_bass_kernel_spmd

# ----------------------------------------------------------------- problem dims
BATCH = 8192
IN_FEATURES = 3072
NCORES = 8
BPC = BATCH // NCORES          # 1024 batch rows per core
NOUT = 10

SHAPES = [((4, 750, 768), (4, 750, 750)),
          ((4, 500, 750), (4, 500, 500)),
          ((4, 250, 500), (4, 250, 250)),
          ((4, 125, 250), (4, 125, 125)),
          ((4, 50, 125), (4, 50, 50)),
          ((4, 25, 50), (4, 25, 25)),
          ((4, 3, 25), (4, 3, 3))]
NLAYERS = 7
NMONARCH = 3                   # layers emitted as 2 block-sparse GEMMs
CS = 2                         # batch column chunks (512 wide)
CW = BPC // CS

F32 = mybir.dt.float32
F16 = mybir.dt.float16
FP8 = mybir.dt.float8e4

CFG = _os.environ.get("KERNEL_CFG", "A")
NFP8 = {"A": 2, "B": 1, "C": 0}[CFG]
LDT = [FP8 if i < NFP8 else F16 for i in range(NLAYERS)]


# ------------------------------------------------------------------ layouts
class Layout:
    """Placement of 4 feature blocks of size Sb into 128-partition tiles."""

    @classmethod
    def from_positions(cls, Sb, ntiles, feat_tile, feat_row):
        self = object.__new__(cls)
        self.Sb = Sb
        self.ntiles = ntiles
        self.feat_tile = feat_tile
        self.feat_row = feat_row
        self._finish()
        return self

    def _finish(self):
        self.valid = np.zeros(self.ntiles, np.int64)
        for k in range(4):
            for t, r in zip(self.feat_tile[k], self.feat_row[k]):
                self.valid[t] = max(self.valid[t], r + 1)
        self.grow = [self.feat_tile[k] * 128 + self.feat_row[k]
                     for k in range(4)]
        self.tiles_of_block = [sorted(set(self.feat_tile[k].tolist()))
                               for k in range(4)]


def simple_layout(Sb):
    if Sb >= 128:
        cpb = (Sb + 127) // 128
        ft, fr = [], []
        for k in range(4):
            i = np.arange(Sb)
            ft.append(k * cpb + i // 128)
            fr.append(i % 128)
        return Layout.from_positions(Sb, 4 * cpb, ft, fr)
    stride = ((Sb + 31) // 32) * 32
    bpt = max(1, 128 // stride)
    ntiles = (4 + bpt - 1) // bpt
    ft, fr = [], []
    for k in range(4):
        i = np.arange(Sb)
        ft.append(np.full(Sb, k // bpt, np.int64))
        fr.append((k % bpt) * stride + i)
    return Layout.from_positions(Sb, ntiles, ft, fr)


def grouped_mid_layout(R, Q):
    """Mid layout with features regrouped by input block k (R >= 125)."""
    cpb = max(1, (R + 127) // 128)
    block_rows = cpb * 128
    Gp = block_rows // 4
    ft, fr = [], []
    for l in range(4):
        rs = np.arange(R)
        ks = (4 * rs + l) // Q
        pos = np.empty(R, np.int64)
        for k in range(4):
            idx = rs[ks == k]
            assert len(idx) <= Gp
            pos[idx] = k * Gp + np.arange(len(idx))
        ft.append(l * cpb + pos // 128)
        fr.append(pos % 128)
    return Layout.from_positions(R, 4 * cpb, ft, fr)


def dense_mats(w1, w2, lin, lmid, lout, Q):
    """Dense effective W1full [in_ext, mid_ext], W2full [mid_ext, out_ext]."""
    _, S, R = w2.shape
    W1full = np.zeros((lin.ntiles * 128, lmid.ntiles * 128), np.float32)
    W2full = np.zeros((lmid.ntiles * 128, lout.ntiles * 128), np.float32)
    for l in range(4):
        js = 4 * np.arange(R) + l
        ks, qs = js // Q, js % Q
        mcols = lmid.grow[l]
        for k in range(4):
            sel = np.where(ks == k)[0]
            if len(sel) == 0:
                continue
            W1full[np.ix_(lin.grow[k], mcols[sel])] = \
                np.ascontiguousarray(w1[k, qs[sel], :].T)
        W2full[np.ix_(lmid.grow[l], lout.grow[l])] = \
            np.ascontiguousarray(w2[l].T)
    return W1full, W2full


class LayerPlan:
    """Monarch layer as two block-sparse GEMMs (layers 1..NMONARCH)."""

    def __init__(self, li, w1_shape, w2_shape, in_layout):
        _, Q, P = w1_shape
        _, S, R = w2_shape
        self.li, self.P, self.Q, self.R, self.S = li, P, Q, R, S
        self.lin = in_layout
        self.lmid = grouped_mid_layout(R, Q)
        self.lout = simple_layout(S)
        self.fused = False
        self.dr = LDT[li] == FP8
        self._build()

    def _build(self):
        Q, R, S = self.Q, self.R, self.S
        ks_of = [(4 * np.arange(R) + l) // Q for l in range(4)]

        need1 = {}
        for l in range(4):
            for r in range(R):
                mt = int(self.lmid.feat_tile[l][r])
                k = int(ks_of[l][r])
                need1.setdefault(mt, set()).update(self.lin.tiles_of_block[k])
        self.g1_chains = {mt: sorted(its) for mt, its in need1.items()}
        self.w1_blocks = [(mt, it) for mt in sorted(need1)
                          for it in self.g1_chains[mt]]
        self.w1_block_of = {p: i for i, p in enumerate(self.w1_blocks)}

        need2 = {}
        for l in range(4):
            for s in range(S):
                ot = int(self.lout.feat_tile[l][s])
                need2.setdefault(ot, set()).update(self.lmid.tiles_of_block[l])
        self.g2_chains = {ot: sorted(mts) for ot, mts in need2.items()}
        self.w2_blocks = [(ot, mt) for ot in sorted(need2)
                          for mt in self.g2_chains[ot]]
        self.w2_block_of = {p: i for i, p in enumerate(self.w2_blocks)}

        self.mid_tiles_of_l = [self.lmid.tiles_of_block[l] for l in range(4)]
        self.out_tiles_of_l = [self.lout.tiles_of_block[l] for l in range(4)]
        if self.dr:
            for mt, its in self.g1_chains.items():
                # pairs must not straddle input blocks (rhs tiles must be
                # adjacent in SBUF); every k-segment here is even-length
                for k in sorted({t // len(self.lin.tiles_of_block[0])
                                 for t in its}):
                    seg = [t for t in its if t in self.lin.tiles_of_block[k]]
                    assert len(seg) % 2 == 0
            for ot, mts in self.g2_chains.items():
                assert len(mts) % 2 == 0

        # w1 block ranges per mid-block l (for per-l weight tiles)
        self.w1_range_of_l = []
        for l in range(4):
            mts = self.mid_tiles_of_l[l]
            idxs = [i for i, (mt, _) in enumerate(self.w1_blocks) if mt in mts]
            assert idxs == list(range(idxs[0], idxs[0] + len(idxs)))
            self.w1_range_of_l.append((idxs[0], len(idxs)))
        self.w2_range_of_l = []
        for l in range(4):
            ots = self.out_tiles_of_l[l]
            idxs = [i for i, (ot, _) in enumerate(self.w2_blocks) if ot in ots]
            assert idxs == list(range(idxs[0], idxs[0] + len(idxs)))
            self.w2_range_of_l.append((idxs[0], len(idxs)))

    def build_weights(self, w1, w2):
        W1full, W2full = dense_mats(w1, w2, self.lin, self.lmid, self.lout,
                                    self.Q)
        W1m = np.zeros((128, 128 * len(self.w1_blocks)), np.float32)
        for i, (mt, it) in enumerate(self.w1_blocks):
            W1m[:, i * 128:(i + 1) * 128] = \
                W1full[it * 128:(it + 1) * 128, mt * 128:(mt + 1) * 128]
        W2m = np.zeros((128, 128 * len(self.w2_blocks)), np.float32)
        for i, (ot, mt) in enumerate(self.w2_blocks):
            W2m[:, i * 128:(i + 1) * 128] = \
                W2full[mt * 128:(mt + 1) * 128, ot * 128:(ot + 1) * 128]
        return W1m, W2m


class FusedPlan:
    """Layers 4-6: one dense GEMM over the effective layer matrix."""

    def __init__(self, li, w1_shape, w2_shape, in_layout):
        _, Q, P = w1_shape
        _, S, R = w2_shape
        self.li, self.Q, self.R, self.S = li, Q, R, S
        self.lin = in_layout
        self.lmid = simple_layout(R)       # host-only intermediate
        self.lout = simple_layout(S)
        self.fused = True
        self.dr = False
        its = list(range(self.lin.ntiles))
        self.chains = {ot: its for ot in range(self.lout.ntiles)}
        self.blocks = [(ot, it) for ot in range(self.lout.ntiles)
                       for it in its]
        self.block_of = {p: i for i, p in enumerate(self.blocks)}

    def build_weights(self, w1, w2):
        W1full, W2full = dense_mats(w1, w2, self.lin, self.lmid, self.lout,
                                    self.Q)
        E = W1full @ W2full
        Em = np.zeros((128, 128 * len(self.blocks)), np.float32)
        for i, (ot, it) in enumerate(self.blocks):
            Em[:, i * 128:(i + 1) * 128] = \
                E[it * 128:(it + 1) * 128, ot * 128:(ot + 1) * 128]
        return Em


class FinalPlan:
    """Layer 7: dense effective [in_ext, 12], consumed batch-major."""

    def __init__(self, li, w1_shape, w2_shape, in_layout):
        _, Q, P = w1_shape
        _, S, R = w2_shape
        self.li, self.Q, self.R, self.S = li, Q, R, S
        self.lin = in_layout
        assert self.lin.ntiles == 1
        self.lmid = simple_layout(R)
        self.lout = simple_layout(S)
        self.fused = True
        self.dr = False

    def build_weights(self, w1, w2):
        W1full, W2full = dense_mats(w1, w2, self.lin, self.lmid, self.lout,
                                    self.Q)
        E = W1full @ W2full
        cols = [self.lout.grow[l][s] for l in range(4) for s in range(3)]
        return np.ascontiguousarray(E[:, cols[:NOUT]])   # [128, 10]


def build_plans():
    plans = []
    lin = simple_layout(SHAPES[0][0][2])
    for i, (s1, s2) in enumerate(SHAPES):
        if i < NMONARCH:
            pl = LayerPlan(i, s1, s2, lin)
        elif i < NLAYERS - 1:
            pl = FusedPlan(i, s1, s2, lin)
        else:
            pl = FinalPlan(i, s1, s2, lin)
        plans.append(pl)
        lin = pl.lout
    return plans


# ------------------------------------------------------------------ bass program
def build_program(plans):
    nc = bacc_mod.Bacc()

    p0 = plans[0]
    XT_T = p0.lin.ntiles                     # 24 input tiles
    KT = XT_T // 4                           # tiles per input block
    xT = nc.dram_tensor("xT", [XT_T, 128, BPC], LDT[0], kind="ExternalInput")
    w1d, w2d, wfd = {}, {}, {}
    for i, pl in enumerate(plans):
        if not pl.fused:
            w1d[i] = nc.dram_tensor(f"w1c_{i}", [128, 128 * len(pl.w1_blocks)],
                                    LDT[i], kind="ExternalInput")
            w2d[i] = nc.dram_tensor(f"w2c_{i}", [128, 128 * len(pl.w2_blocks)],
                                    LDT[i], kind="ExternalInput")
        elif i < NLAYERS - 1:
            wfd[i] = nc.dram_tensor(f"wc_{i}", [128, 128 * len(pl.blocks)],
                                    F16, kind="ExternalInput")
        else:
            wfd[i] = nc.dram_tensor(f"wc_{i}", [128, NOUT], F16,
                                    kind="ExternalInput")
    y = nc.dram_tensor("y", [BPC, NOUT], F32, kind="ExternalOutput")

    with tile.TileContext(nc) as tc:
        with (
            tc.tile_pool(name="sb", bufs=1) as sb,
            tc.tile_pool(name="ps", bufs=1, space="PSUM") as ps,
        ):
            evict_flip = [0]

            def evict(dst_ap, src_ap, relu):
                e = evict_flip[0] = evict_flip[0] ^ 1
                if relu:
                    if e:
                        nc.vector.tensor_scalar_max(dst_ap, src_ap, 0.0)
                    else:
                        nc.scalar.activation(dst_ap, src_ap,
                                             mybir.ActivationFunctionType.Relu)
                else:
                    if e:
                        nc.vector.tensor_copy(dst_ap, src_ap)
                    else:
                        nc.scalar.copy(dst_ap, src_ap)

            def zero_pads(tile_ap, layout, loc_tiles, cols):
                """memset pad rows of partial tiles (needed under DoubleRow).

                Engines require a 32-aligned partition base, so zero from the
                aligned floor; the eviction that follows overwrites the
                overlap with real data (WAW ordering handles it)."""
                locs = [(loc, (int(layout.valid[t]) // 32) * 32)
                        for loc, t in enumerate(loc_tiles)
                        if int(layout.valid[t]) < 128]
                if not locs:
                    return
                v0s = {v0 for _, v0 in locs}
                if len(v0s) == 1 and [loc for loc, _ in locs] == \
                        list(range(locs[0][0], locs[0][0] + len(locs))):
                    v0 = v0s.pop()
                    l0 = locs[0][0]
                    nc.gpsimd.memset(
                        tile_ap[v0:128, l0:l0 + len(locs), 0:cols], 0)
                else:
                    for loc, v0 in locs:
                        nc.gpsimd.memset(tile_ap[v0:128, loc, 0:cols], 0)

            # =========================== layer 1 ===========================
            # x chunk tiles, one DMA each: hx[k][cs] = [128, KT, CW]
            hx = [[sb.tile([128, KT, CW], LDT[0], name=f"x_{k}_{cs}",
                           tag=f"hx{k}{cs}") for cs in range(CS)]
                  for k in range(4)]

            def dma_x(k, cs):
                nc.sync.dma_start(
                    out=hx[k][cs][:, :, :],
                    in_=xT[k * KT:(k + 1) * KT, :, cs * CW:(cs + 1) * CW]
                    .rearrange("t p n -> p t n"))

            G1 = 1 if LDT[0] == FP8 else 2    # weight groups (SBUF pressure)
            w1t, w2t = {}, {}

            def dma_w1_l(l):
                b0, nb = p0.w1_range_of_l[l]
                tg = l if G1 == 1 else l % 2
                w1t[l] = sb.tile([128, nb, 128], LDT[0], name=f"w1L0_{l}",
                                 tag=f"w1L0_{tg}")
                nc.sync.dma_start(
                    out=w1t[l][:, :, :],
                    in_=w1d[0][:, b0 * 128:(b0 + nb) * 128]
                    .rearrange("p (n m) -> p n m", m=128))

            def dma_w2_l(l):
                b0, nb = p0.w2_range_of_l[l]
                tg = l if G1 == 1 else l % 2
                w2t[l] = sb.tile([128, nb, 128], LDT[0], name=f"w2L0_{l}",
                                 tag=f"w2L0_{tg}")
                nc.sync.dma_start(
                    out=w2t[l][:, :, :],
                    in_=w2d[0][:, b0 * 128:(b0 + nb) * 128]
                    .rearrange("p (n m) -> p n m", m=128))

            h2 = sb.tile([128, p0.lout.ntiles, BPC], LDT[1], name="h_1",
                         tag="hB")
            if plans[1].dr:
                zero_pads(h2, p0.lout, range(p0.lout.ntiles), BPC)

            def l1_g1_chain(l, mt, mtloc, midl, cs):
                V = int(p0.lmid.valid[mt])
                its = p0.g1_chains[mt]
                b0_l = p0.w1_range_of_l[l][0]
                pm = ps.tile([128, CW], F32, name="pm_0", tag="pmid", bufs=3)
                if p0.dr:
                    j = 0
                    first = True
                    while j < len(its):
                        t0 = its[j]
                        k = t0 // KT
                        b = p0.w1_block_of[(mt, t0)] - b0_l
                        lhsT = w1t[l][:, b:b + 2, 0:V]
                        rhs = hx[k][cs][:, (t0 % KT):(t0 % KT) + 2, :]
                        nc.tensor.matmul(pm[0:V, :], lhsT, rhs,
                                         start=first, stop=(j + 2 >= len(its)),
                                         perf_mode=mybir.MatmulPerfMode.DoubleRow)
                        first = False
                        j += 2
                else:
                    for j, it in enumerate(its):
                        ln = int(p0.lin.valid[it])
                        k = it // KT
                        b = p0.w1_block_of[(mt, it)] - b0_l
                        lhsT = w1t[l][0:ln, b, 0:V]
                        rhs = hx[k][cs][0:ln, it % KT, :]
                        nc.tensor.matmul(pm[0:V, :], lhsT, rhs,
                                         start=(j == 0),
                                         stop=(j == len(its) - 1))
                evict(midl[0:V, mtloc, :], pm[0:V, :], relu=False)

            def l1_g2_chain(l, ot, mid_of, cs):
                V = int(p0.lout.valid[ot])
                mts = p0.g2_chains[ot]
                b0_l = p0.w2_range_of_l[l][0]
                po = ps.tile([128, CW], F32, name="po_0", tag="pout", bufs=3)
                if p0.dr:
                    for j in range(0, len(mts), 2):
                        mt = mts[j]
                        b = p0.w2_block_of[(ot, mt)] - b0_l
                        midl, loc = mid_of[mt]
                        lhsT = w2t[l][:, b:b + 2, 0:V]
                        rhs = midl[:, loc:loc + 2, :]
                        nc.tensor.matmul(po[0:V, :], lhsT, rhs,
                                         start=(j == 0),
                                         stop=(j + 2 >= len(mts)),
                                         perf_mode=mybir.MatmulPerfMode.DoubleRow)
                else:
                    for j, mt in enumerate(mts):
                        ln = int(p0.lmid.valid[mt])
                        b = p0.w2_block_of[(ot, mt)] - b0_l
                        midl, loc = mid_of[mt]
                        lhsT = w2t[l][0:ln, b, 0:V]
                        rhs = midl[0:ln, loc, :]
                        nc.tensor.matmul(po[0:V, :], lhsT, rhs,
                                         start=(j == 0),
                                         stop=(j == len(mts) - 1))
                evict(h2[0:V, ot, cs * CW:(cs + 1) * CW], po[0:V, :],
                      relu=True)

            # head DMA interleave + L1 emission (cs-outer, per-l lookahead)
            for g in range(G1):
                ls = list(range(4)) if G1 == 1 else [2 * g, 2 * g + 1]
                if g == 0:
                    dma_x(0, 0)
                    dma_w1_l(ls[0])
                    dma_x(1, 0)
                    dma_w1_l(ls[1])
                    dma_x(2, 0)
                    dma_x(3, 0)
                    dma_w2_l(ls[0])
                    if G1 == 1:
                        dma_w1_l(2)
                    dma_x(0, 1)
                    dma_x(1, 1)
                    dma_w2_l(ls[1])
                    if G1 == 1:
                        dma_w1_l(3)
                    dma_x(2, 1)
                    dma_x(3, 1)
                    if G1 == 1:
                        dma_w2_l(2)
                        dma_w2_l(3)
                else:
                    dma_w1_l(2)
                    dma_w1_l(3)
                    dma_w2_l(2)
                    dma_w2_l(3)

                for cs in range(CS):
                    mid_of = {}
                    pend = None
                    for l in ls:
                        mts_l = p0.mid_tiles_of_l[l]
                        midl = sb.tile([128, len(mts_l), CW], LDT[0],
                                       name=f"mid_0_{l}_{cs}", tag="midb",
                                       bufs=3)
                        if p0.dr:
                            zero_pads(midl, p0.lmid, mts_l, CW)
                        for loc, mt in enumerate(mts_l):
                            mid_of[mt] = (midl, loc)
                            l1_g1_chain(l, mt, loc, midl, cs)
                        if pend is not None:
                            for ot in p0.out_tiles_of_l[pend]:
                                l1_g2_chain(pend, ot, mid_of, cs)
                        pend = l
                    for ot in p0.out_tiles_of_l[pend]:
                        l1_g2_chain(pend, ot, mid_of, cs)

            # ====================== layers 2..NMONARCH ======================
            hin = h2
            for li in range(1, NMONARCH):
                pl = plans[li]
                w1sb = sb.tile([128, len(pl.w1_blocks), 128], LDT[li],
                               name=f"w1sb_{li}", tag="w1", bufs=2)
                nc.sync.dma_start(
                    out=w1sb[:, :, :],
                    in_=w1d[li][:, :].rearrange("p (n m) -> p n m", m=128))
                w2sb = sb.tile([128, len(pl.w2_blocks), 128], LDT[li],
                               name=f"w2sb_{li}", tag="w2", bufs=2)
                nc.sync.dma_start(
                    out=w2sb[:, :, :],
                    in_=w2d[li][:, :].rearrange("p (n m) -> p n m", m=128))

                hnext = sb.tile([128, pl.lout.ntiles, BPC], LDT[li + 1],
                                name=f"h_{li + 1}",
                                tag="hA" if li % 2 == 1 else "hB")
                if plans[li + 1].dr:
                    zero_pads(hnext, pl.lout, range(pl.lout.ntiles), BPC)

                def g1_tile(mt, mtloc, midl, cs):
                    V = int(pl.lmid.valid[mt])
                    its = pl.g1_chains[mt]
                    c0 = cs * CW
                    pm = ps.tile([128, CW], F32, name=f"pm_{li}", tag="pmid",
                                 bufs=3)
                    if pl.dr:
                        for j in range(0, len(its), 2):
                            it = its[j]
                            b = pl.w1_block_of[(mt, it)]
                            lhsT = w1sb[:, b:b + 2, 0:V]
                            rhs = hin[:, it:it + 2, c0:c0 + CW]
                            nc.tensor.matmul(
                                pm[0:V, :], lhsT, rhs,
                                start=(j == 0), stop=(j + 2 >= len(its)),
                                perf_mode=mybir.MatmulPerfMode.DoubleRow)
                    else:
                        for j, it in enumerate(its):
                            ln = int(pl.lin.valid[it])
                            b = pl.w1_block_of[(mt, it)]
                            lhsT = w1sb[0:ln, b, 0:V]
                            rhs = hin[0:ln, it, c0:c0 + CW]
                            nc.tensor.matmul(pm[0:V, :], lhsT, rhs,
                                             start=(j == 0),
                                             stop=(j == len(its) - 1))
                    evict(midl[0:V, mtloc, c0:c0 + CW], pm[0:V, :],
                          relu=False)

                def g2_tile(ot, mid_of, cs):
                    V = int(pl.lout.valid[ot])
                    mts = pl.g2_chains[ot]
                    c0 = cs * CW
                    po = ps.tile([128, CW], F32, name=f"po_{li}", tag="pout",
                                 bufs=3)
                    if pl.dr:
                        for j in range(0, len(mts), 2):
                            mt = mts[j]
                            b = pl.w2_block_of[(ot, mt)]
                            midl, loc = mid_of[mt]
                            lhsT = w2sb[:, b:b + 2, 0:V]
                            rhs = midl[:, loc:loc + 2, c0:c0 + CW]
                            nc.tensor.matmul(
                                po[0:V, :], lhsT, rhs,
                                start=(j == 0), stop=(j + 2 >= len(mts)),
                                perf_mode=mybir.MatmulPerfMode.DoubleRow)
                    else:
                        for j, mt in enumerate(mts):
                            ln = int(pl.lmid.valid[mt])
                            b = pl.w2_block_of[(ot, mt)]
                            midl, loc = mid_of[mt]
                            lhsT = w2sb[0:ln, b, 0:V]
                            rhs = midl[0:ln, loc, c0:c0 + CW]
                            nc.tensor.matmul(po[0:V, :], lhsT, rhs,
                                             start=(j == 0),
                                             stop=(j == len(mts) - 1))
                    evict(hnext[0:V, ot, c0:c0 + CW], po[0:V, :], relu=True)

                mid_of = {}
                pend = None
                for l in range(4):
                    mts_l = pl.mid_tiles_of_l[l]
                    midl = sb.tile([128, len(mts_l), BPC], LDT[li],
                                   name=f"mid_{li}_{l}", tag="midb2", bufs=2)
                    if pl.dr:
                        zero_pads(midl, pl.lmid, mts_l, BPC)
                    for loc, mt in enumerate(mts_l):
                        mid_of[mt] = (midl, loc)
                        for cs in range(CS):
                            g1_tile(mt, loc, midl, cs)
                    if pend is not None:
                        for ot in pl.out_tiles_of_l[pend]:
                            for cs in range(CS):
                                g2_tile(ot, mid_of, cs)
                    pend = l
                for ot in pl.out_tiles_of_l[pend]:
                    for cs in range(CS):
                        g2_tile(ot, mid_of, cs)
                hin = hnext

            # ======================= fused layers 4-6 =======================
            for li in range(NMONARCH, NLAYERS - 1):
                pl = plans[li]
                wsb = sb.tile([128, len(pl.blocks), 128], F16,
                              name=f"wsb_{li}", tag="w1", bufs=2)
                nc.sync.dma_start(
                    out=wsb[:, :, :],
                    in_=wfd[li][:, :].rearrange("p (n m) -> p n m", m=128))
                hnext = sb.tile([128, pl.lout.ntiles, BPC], F16,
                                name=f"h_{li + 1}",
                                tag="hA" if li % 2 == 1 else "hB")
                for cs in range(CS):
                    c0 = cs * CW
                    for ot in range(pl.lout.ntiles):
                        V = int(pl.lout.valid[ot])
                        its = pl.chains[ot]
                        po = ps.tile([128, CW], F32, name=f"po_{li}",
                                     tag="pout", bufs=3)
                        for j, it in enumerate(its):
                            ln = int(pl.lin.valid[it])
                            b = pl.block_of[(ot, it)]
                            nc.tensor.matmul(po[0:V, :], wsb[0:ln, b, 0:V],
                                             hin[0:ln, it, c0:c0 + CW],
                                             start=(j == 0),
                                             stop=(j == len(its) - 1))
                        evict(hnext[0:V, ot, c0:c0 + CW], po[0:V, :],
                              relu=True)
                hin = hnext

            # ============== layer 7 (batch-major) + log_softmax ==============
            pf_pl = plans[NLAYERS - 1]
            VF = int(pf_pl.lin.valid[0])
            w7sb = sb.tile([128, NOUT], F16, name="w7sb", tag="w7")
            nc.sync.dma_start(out=w7sb[:, :], in_=wfd[NLAYERS - 1][:, :])

            nchunk = BPC // 128
            esum = sb.tile([128, nchunk], F32, name="esum", tag="esum")
            tlog = sb.tile([128, nchunk, NOUT], F32, name="tlog", tag="tlog")
            osb = sb.tile([128, nchunk, NOUT], F32, name="osb", tag="osb")
            lse = sb.tile([128, nchunk], F32, name="lse", tag="lse")
            for bc in range(nchunk):
                pf = ps.tile([128, NOUT], F32, name="pfin", tag="pfin",
                             bufs=2)
                nc.tensor.matmul(pf[:, :],
                                 hin[0:VF, 0, bc * 128:(bc + 1) * 128],
                                 w7sb[0:VF, :], start=True, stop=True)
                esb = sb.tile([128, NOUT], F32, name="esb", tag="esb",
                              bufs=2)
                nc.scalar.activation(esb, pf,
                                     mybir.ActivationFunctionType.Exp,
                                     accum_out=esum[:, bc:bc + 1])
                nc.vector.tensor_copy(tlog[:, bc, :], pf)
            nc.scalar.activation(lse, esum, mybir.ActivationFunctionType.Ln)
            for bc in range(nchunk):
                nc.vector.tensor_scalar_sub(osb[:, bc, :], tlog[:, bc, :],
                                            lse[:, bc:bc + 1])
                nc.sync.dma_start(out=y[bc * 128:(bc + 1) * 128, :],
                                  in_=osb[:, bc, :])
    nc.finalize()
    return nc


# --------------------------------------------------- numpy model of the schedule
def numpy_forward(plans, weights, xT):
    """Mirror the device schedule (incl. quantization) for validation."""
    import ml_dtypes

    def npdt(dt):
        return mybir.dt.np(dt)

    B = xT.shape[1]
    h = np.zeros((plans[0].lin.ntiles * 128, B), np.float32)
    h[:xT.shape[0]] = xT.astype(npdt(LDT[0])).astype(np.float32)
    for pl in plans[:NLAYERS - 1]:
        li = pl.li
        if not pl.fused:
            W1m, W2m = weights[li]
            W1m = W1m.astype(np.float32)
            W2m = W2m.astype(np.float32)
            mid = np.zeros((pl.lmid.ntiles * 128, B), np.float32)
            for mt, its in pl.g1_chains.items():
                V = int(pl.lmid.valid[mt])
                acc = np.zeros((V, B), np.float32)
                for it in its:
                    ln = int(pl.lin.valid[it])
                    b = pl.w1_block_of[(mt, it)]
                    acc += W1m[0:ln, b * 128:b * 128 + V].T @ \
                        h[it * 128: it * 128 + ln]
                mid[mt * 128: mt * 128 + V] = acc
            mid = mid.astype(npdt(LDT[li])).astype(np.float32)
            out = np.zeros((pl.lout.ntiles * 128, B), np.float32)
            for ot, mts in pl.g2_chains.items():
                V = int(pl.lout.valid[ot])
                acc = np.zeros((V, B), np.float32)
                for mt in mts:
                    ln = int(pl.lmid.valid[mt])
                    b = pl.w2_block_of[(ot, mt)]
                    acc += W2m[0:ln, b * 128:b * 128 + V].T @ \
                        mid[mt * 128: mt * 128 + ln]
                out[ot * 128: ot * 128 + V] = acc
        else:
            Em = weights[li].astype(np.float32)
            out = np.zeros((pl.lout.ntiles * 128, B), np.float32)
            for ot, its in pl.chains.items():
                V = int(pl.lout.valid[ot])
                acc = np.zeros((V, B), np.float32)
                for it in its:
                    ln = int(pl.lin.valid[it])
                    b = pl.block_of[(ot, it)]
                    acc += Em[0:ln, b * 128:b * 128 + V].T @ \
                        h[it * 128: it * 128 + ln]
                out[ot * 128: ot * 128 + V] = acc
        out = np.maximum(out, 0.0)
        h = out.astype(npdt(LDT[li + 1])).astype(np.float32)
    E7 = weights[NLAYERS - 1].astype(np.float32)
    VF = int(plans[NLAYERS - 1].lin.valid[0])
    logits = (E7[0:VF, :].T @ h[0:VF]).T
    S = np.exp(logits).sum(axis=1, keepdims=True)
    return logits - np.log(S)


# ------------------------------------------------------------------ entry point
def _prep_inputs(inputs, plans):
    x = np.ascontiguousarray(np.asarray(inputs["x"], dtype=np.float32))
    shared = {}
    for i, pl in enumerate(plans):
        w1 = np.asarray(inputs[f"w1_{i + 1}"], dtype=np.float32)
        w2 = np.asarray(inputs[f"w2_{i + 1}"], dtype=np.float32)
        if not pl.fused:
            np_dt = mybir.dt.np(LDT[i])
            W1m, W2m = pl.build_weights(w1, w2)
            shared[f"w1c_{i}"] = np.ascontiguousarray(W1m.astype(np_dt))
            shared[f"w2c_{i}"] = np.ascontiguousarray(W2m.astype(np_dt))
        else:
            Em = pl.build_weights(w1, w2)
            shared[f"wc_{i}"] = np.ascontiguousarray(
                Em.astype(mybir.dt.np(F16)))
    np_x = mybir.dt.np(LDT[0])
    in_maps = []
    for c in range(NCORES):
        m = dict(shared)
        xc = x[c * BPC:(c + 1) * BPC].T.astype(np_x)
        m["xT"] = np.ascontiguousarray(
            xc.reshape(plans[0].lin.ntiles, 128, BPC))
        in_maps.append(m)
    return in_maps


def _run(inputs, trace=False, **spmd_kwargs):
    plans = build_plans()
    in_maps = _prep_inputs(inputs, plans)
    nc = build_program(plans)
    res = run_bass_kernel_spmd(nc, in_maps, core_ids=list(range(NCORES)),
                               trace=trace, **spmd_kwargs)
    out = np.concatenate([r["y"] for r in res.results], axis=0)
    return out.astype(np.float32), res


def kernel(**inputs):
    out, _ = _run(inputs, trace=False)
    return out


# revision 8
# speedup vs baseline: 1.0460x; 1.0028x over previous
"""Trainium2 Bass kernel for the CIFAR10 Monarch MLP (7 monarch layers + log_softmax).

Strategy
--------
Pure data parallel over 8 NeuronCores: each core takes a 1024-row batch shard;
weights are replicated. Activations are feature-major in SBUF
([feature-tile partitions, batch free dim]), fully SBUF-resident; only x,
weights and final log-probs cross HBM.

Performance structure (vs the v1 baseline):
- Layers 1-2 run in fp8(e4m3) with DoubleRow matmuls: each PE instruction
  contracts 256 rows (two 128-row tiles packed in the stationary/moving
  operands), 2x the bf16 rate. Partial 128-row tiles are zero-padded
  (host-side for weights, gpsimd memset for activations) so pairs can always
  contract over a full 128 partitions.
- Layers 3+ run in fp16 (same PE rate as bf16, 8x finer mantissa, which
  leaves the error budget to the fp8 layers).
- Layers 4-6 are folded into single dense GEMMs (effective W1*P*W2 built on
  the host): the block-diag structure is too fine for 128-wide tiles there,
  so dense has fewer matmuls and no mid eviction on the critical path.
- Layer 7 + log_softmax: logits are produced *batch-major* by using the
  activations as the stationary operand (out[batch,12] = h6_chunk.T @ W7),
  which kills the transposes; softmax skips max-subtraction (|logit| < 1)
  and runs two-pass (all Exp accumulations, then one Ln) so the scalar
  engine loads each activation table exactly once.
- Head: x and layer-1 weights are DMAed in interleaved (k-block, col-half)
  chunks ordered so the first matmul can start ~10us in, instead of waiting
  ~38us for everything.

Config via KERNEL_CFG: "A" = fp8 L1+L2 (default), "B" = fp8 L1 only,
"C" = all fp16.
"""

import os as _os

import numpy as np

import concourse.bacc as bacc_mod
import concourse.mybir as mybir
import concourse.tile as tile
from concourse.bass_utils import run_bass_kernel_spmd

# ----------------------------------------------------------------- problem dims
BATCH = 8192
IN_FEATURES = 3072
NCORES = 8
BPC = BATCH // NCORES          # 1024 batch rows per core
NOUT = 10

SHAPES = [((4, 750, 768), (4, 750, 750)),
          ((4, 500, 750), (4, 500, 500)),
          ((4, 250, 500), (4, 250, 250)),
          ((4, 125, 250), (4, 125, 125)),
          ((4, 50, 125), (4, 50, 50)),
          ((4, 25, 50), (4, 25, 25)),
          ((4, 3, 25), (4, 3, 3))]
NLAYERS = 7
NMONARCH = 3                   # layers emitted as 2 block-sparse GEMMs
CS = 2                         # batch column chunks (512 wide)
CW = BPC // CS

F32 = mybir.dt.float32
F16 = mybir.dt.float16
FP8 = mybir.dt.float8e4

CFG = _os.environ.get("KERNEL_CFG", "A")
NFP8 = {"A": 2, "B": 1, "C": 0}[CFG]
LDT = [FP8 if i < NFP8 else F16 for i in range(NLAYERS)]


# ------------------------------------------------------------------ layouts
class Layout:
    """Placement of 4 feature blocks of size Sb into 128-partition tiles."""

    @classmethod
    def from_positions(cls, Sb, ntiles, feat_tile, feat_row):
        self = object.__new__(cls)
        self.Sb = Sb
        self.ntiles = ntiles
        self.feat_tile = feat_tile
        self.feat_row = feat_row
        self._finish()
        return self

    def _finish(self):
        self.valid = np.zeros(self.ntiles, np.int64)
        for k in range(4):
            for t, r in zip(self.feat_tile[k], self.feat_row[k]):
                self.valid[t] = max(self.valid[t], r + 1)
        self.grow = [self.feat_tile[k] * 128 + self.feat_row[k]
                     for k in range(4)]
        self.tiles_of_block = [sorted(set(self.feat_tile[k].tolist()))
                               for k in range(4)]


def simple_layout(Sb):
    if Sb >= 128:
        cpb = (Sb + 127) // 128
        ft, fr = [], []
        for k in range(4):
            i = np.arange(Sb)
            ft.append(k * cpb + i // 128)
            fr.append(i % 128)
        return Layout.from_positions(Sb, 4 * cpb, ft, fr)
    stride = ((Sb + 31) // 32) * 32
    bpt = max(1, 128 // stride)
    ntiles = (4 + bpt - 1) // bpt
    ft, fr = [], []
    for k in range(4):
        i = np.arange(Sb)
        ft.append(np.full(Sb, k // bpt, np.int64))
        fr.append((k % bpt) * stride + i)
    return Layout.from_positions(Sb, ntiles, ft, fr)


def grouped_mid_layout(R, Q):
    """Mid layout with features regrouped by input block k (R >= 125)."""
    cpb = max(1, (R + 127) // 128)
    block_rows = cpb * 128
    Gp = block_rows // 4
    ft, fr = [], []
    for l in range(4):
        rs = np.arange(R)
        ks = (4 * rs + l) // Q
        pos = np.empty(R, np.int64)
        for k in range(4):
            idx = rs[ks == k]
            assert len(idx) <= Gp
            pos[idx] = k * Gp + np.arange(len(idx))
        ft.append(l * cpb + pos // 128)
        fr.append(pos % 128)
    return Layout.from_positions(R, 4 * cpb, ft, fr)


def dense_mats(w1, w2, lin, lmid, lout, Q):
    """Dense effective W1full [in_ext, mid_ext], W2full [mid_ext, out_ext]."""
    _, S, R = w2.shape
    W1full = np.zeros((lin.ntiles * 128, lmid.ntiles * 128), np.float32)
    W2full = np.zeros((lmid.ntiles * 128, lout.ntiles * 128), np.float32)
    for l in range(4):
        js = 4 * np.arange(R) + l
        ks, qs = js // Q, js % Q
        mcols = lmid.grow[l]
        for k in range(4):
            sel = np.where(ks == k)[0]
            if len(sel) == 0:
                continue
            W1full[np.ix_(lin.grow[k], mcols[sel])] = \
                np.ascontiguousarray(w1[k, qs[sel], :].T)
        W2full[np.ix_(lmid.grow[l], lout.grow[l])] = \
            np.ascontiguousarray(w2[l].T)
    return W1full, W2full


class LayerPlan:
    """Monarch layer as two block-sparse GEMMs (layers 1..NMONARCH)."""

    def __init__(self, li, w1_shape, w2_shape, in_layout):
        _, Q, P = w1_shape
        _, S, R = w2_shape
        self.li, self.P, self.Q, self.R, self.S = li, P, Q, R, S
        self.lin = in_layout
        self.lmid = grouped_mid_layout(R, Q)
        self.lout = simple_layout(S)
        self.fused = False
        self.dr = LDT[li] == FP8
        self._build()

    def _build(self):
        Q, R, S = self.Q, self.R, self.S
        ks_of = [(4 * np.arange(R) + l) // Q for l in range(4)]

        need1 = {}
        for l in range(4):
            for r in range(R):
                mt = int(self.lmid.feat_tile[l][r])
                k = int(ks_of[l][r])
                need1.setdefault(mt, set()).update(self.lin.tiles_of_block[k])
        self.g1_chains = {mt: sorted(its) for mt, its in need1.items()}
        self.w1_blocks = [(mt, it) for mt in sorted(need1)
                          for it in self.g1_chains[mt]]
        self.w1_block_of = {p: i for i, p in enumerate(self.w1_blocks)}

        need2 = {}
        for l in range(4):
            for s in range(S):
                ot = int(self.lout.feat_tile[l][s])
                need2.setdefault(ot, set()).update(self.lmid.tiles_of_block[l])
        self.g2_chains = {ot: sorted(mts) for ot, mts in need2.items()}
        self.w2_blocks = [(ot, mt) for ot in sorted(need2)
                          for mt in self.g2_chains[ot]]
        self.w2_block_of = {p: i for i, p in enumerate(self.w2_blocks)}

        self.mid_tiles_of_l = [self.lmid.tiles_of_block[l] for l in range(4)]
        self.out_tiles_of_l = [self.lout.tiles_of_block[l] for l in range(4)]
        if self.dr:
            for mt, its in self.g1_chains.items():
                # pairs must not straddle input blocks (rhs tiles must be
                # adjacent in SBUF); every k-segment here is even-length
                for k in sorted({t // len(self.lin.tiles_of_block[0])
                                 for t in its}):
                    seg = [t for t in its if t in self.lin.tiles_of_block[k]]
                    assert len(seg) % 2 == 0
            for ot, mts in self.g2_chains.items():
                assert len(mts) % 2 == 0

        # w1 block ranges per mid-block l (for per-l weight tiles)
        self.w1_range_of_l = []
        for l in range(4):
            mts = self.mid_tiles_of_l[l]
            idxs = [i for i, (mt, _) in enumerate(self.w1_blocks) if mt in mts]
            assert idxs == list(range(idxs[0], idxs[0] + len(idxs)))
            self.w1_range_of_l.append((idxs[0], len(idxs)))
        self.w2_range_of_l = []
        for l in range(4):
            ots = self.out_tiles_of_l[l]
            idxs = [i for i, (ot, _) in enumerate(self.w2_blocks) if ot in ots]
            assert idxs == list(range(idxs[0], idxs[0] + len(idxs)))
            self.w2_range_of_l.append((idxs[0], len(idxs)))

    def build_weights(self, w1, w2):
        W1full, W2full = dense_mats(w1, w2, self.lin, self.lmid, self.lout,
                                    self.Q)
        W1m = np.zeros((128, 128 * len(self.w1_blocks)), np.float32)
        for i, (mt, it) in enumerate(self.w1_blocks):
            W1m[:, i * 128:(i + 1) * 128] = \
                W1full[it * 128:(it + 1) * 128, mt * 128:(mt + 1) * 128]
        W2m = np.zeros((128, 128 * len(self.w2_blocks)), np.float32)
        for i, (ot, mt) in enumerate(self.w2_blocks):
            W2m[:, i * 128:(i + 1) * 128] = \
                W2full[mt * 128:(mt + 1) * 128, ot * 128:(ot + 1) * 128]
        return W1m, W2m


class FusedPlan:
    """Layers 4-6: one dense GEMM over the effective layer matrix."""

    def __init__(self, li, w1_shape, w2_shape, in_layout):
        _, Q, P = w1_shape
        _, S, R = w2_shape
        self.li, self.Q, self.R, self.S = li, Q, R, S
        self.lin = in_layout
        self.lmid = simple_layout(R)       # host-only intermediate
        self.lout = simple_layout(S)
        self.fused = True
        self.dr = False
        its = list(range(self.lin.ntiles))
        self.chains = {ot: its for ot in range(self.lout.ntiles)}
        self.blocks = [(ot, it) for ot in range(self.lout.ntiles)
                       for it in its]
        self.block_of = {p: i for i, p in enumerate(self.blocks)}

    def build_weights(self, w1, w2):
        W1full, W2full = dense_mats(w1, w2, self.lin, self.lmid, self.lout,
                                    self.Q)
        E = W1full @ W2full
        Em = np.zeros((128, 128 * len(self.blocks)), np.float32)
        for i, (ot, it) in enumerate(self.blocks):
            Em[:, i * 128:(i + 1) * 128] = \
                E[it * 128:(it + 1) * 128, ot * 128:(ot + 1) * 128]
        return Em


class FinalPlan:
    """Layer 7: dense effective [in_ext, 12], consumed batch-major."""

    def __init__(self, li, w1_shape, w2_shape, in_layout):
        _, Q, P = w1_shape
        _, S, R = w2_shape
        self.li, self.Q, self.R, self.S = li, Q, R, S
        self.lin = in_layout
        assert self.lin.ntiles == 1
        self.lmid = simple_layout(R)
        self.lout = simple_layout(S)
        self.fused = True
        self.dr = False

    def build_weights(self, w1, w2):
        W1full, W2full = dense_mats(w1, w2, self.lin, self.lmid, self.lout,
                                    self.Q)
        E = W1full @ W2full
        cols = [self.lout.grow[l][s] for l in range(4) for s in range(3)]
        return np.ascontiguousarray(E[:, cols[:NOUT]])   # [128, 10]


def build_plans():
    plans = []
    lin = simple_layout(SHAPES[0][0][2])
    for i, (s1, s2) in enumerate(SHAPES):
        if i < NMONARCH:
            pl = LayerPlan(i, s1, s2, lin)
        elif i < NLAYERS - 1:
            pl = FusedPlan(i, s1, s2, lin)
        else:
            pl = FinalPlan(i, s1, s2, lin)
        plans.append(pl)
        lin = pl.lout
    return plans


# ------------------------------------------------------------------ bass program
def build_program(plans):
    nc = bacc_mod.Bacc()

    p0 = plans[0]
    XT_T = p0.lin.ntiles                     # 24 input tiles
    KT = XT_T // 4                           # tiles per input block
    xT = nc.dram_tensor("xT", [XT_T, 128, BPC], LDT[0], kind="ExternalInput")
    w1d, w2d, wfd = {}, {}, {}
    for i, pl in enumerate(plans):
        if not pl.fused:
            w1d[i] = nc.dram_tensor(f"w1c_{i}", [128, 128 * len(pl.w1_blocks)],
                                    LDT[i], kind="ExternalInput")
            w2d[i] = nc.dram_tensor(f"w2c_{i}", [128, 128 * len(pl.w2_blocks)],
                                    LDT[i], kind="ExternalInput")
        elif i < NLAYERS - 1:
            wfd[i] = nc.dram_tensor(f"wc_{i}", [128, 128 * len(pl.blocks)],
                                    F16, kind="ExternalInput")
        else:
            wfd[i] = nc.dram_tensor(f"wc_{i}", [128, NOUT], F16,
                                    kind="ExternalInput")
    y = nc.dram_tensor("y", [BPC, NOUT], F32, kind="ExternalOutput")

    with tile.TileContext(nc) as tc:
        with (
            tc.tile_pool(name="sb", bufs=1) as sb,
            tc.tile_pool(name="ps", bufs=1, space="PSUM") as ps,
        ):
            evict_flip = [0]

            def evict(dst_ap, src_ap, relu):
                e = evict_flip[0] = evict_flip[0] ^ 1
                if relu:
                    if e:
                        nc.vector.tensor_scalar_max(dst_ap, src_ap, 0.0)
                    else:
                        nc.scalar.activation(dst_ap, src_ap,
                                             mybir.ActivationFunctionType.Relu)
                else:
                    if e:
                        nc.vector.tensor_copy(dst_ap, src_ap)
                    else:
                        nc.scalar.copy(dst_ap, src_ap)

            def zero_pads(tile_ap, layout, loc_tiles, cols):
                """memset pad rows of partial tiles (needed under DoubleRow).

                Engines require a 32-aligned partition base, so zero from the
                aligned floor; the eviction that follows overwrites the
                overlap with real data (WAW ordering handles it)."""
                locs = [(loc, (int(layout.valid[t]) // 32) * 32)
                        for loc, t in enumerate(loc_tiles)
                        if int(layout.valid[t]) < 128]
                if not locs:
                    return
                v0s = {v0 for _, v0 in locs}
                if len(v0s) == 1 and [loc for loc, _ in locs] == \
                        list(range(locs[0][0], locs[0][0] + len(locs))):
                    v0 = v0s.pop()
                    l0 = locs[0][0]
                    nc.gpsimd.memset(
                        tile_ap[v0:128, l0:l0 + len(locs), 0:cols], 0)
                else:
                    for loc, v0 in locs:
                        nc.gpsimd.memset(tile_ap[v0:128, loc, 0:cols], 0)

            # =========================== layer 1 ===========================
            # x chunk tiles, one DMA each: hx[k][cs] = [128, KT, CW]
            hx = [[sb.tile([128, KT, CW], LDT[0], name=f"x_{k}_{cs}",
                           tag=f"hx{k}{cs}") for cs in range(CS)]
                  for k in range(4)]

            def dma_x(k, cs):
                nc.sync.dma_start(
                    out=hx[k][cs][:, :, :],
                    in_=xT[k * KT:(k + 1) * KT, :, cs * CW:(cs + 1) * CW]
                    .rearrange("t p n -> p t n"))

            G1 = 1 if LDT[0] == FP8 else 2    # weight groups (SBUF pressure)
            w1t, w2t = {}, {}

            def dma_w1_l(l):
                b0, nb = p0.w1_range_of_l[l]
                tg = l if G1 == 1 else l % 2
                w1t[l] = sb.tile([128, nb, 128], LDT[0], name=f"w1L0_{l}",
                                 tag=f"w1L0_{tg}")
                nc.sync.dma_start(
                    out=w1t[l][:, :, :],
                    in_=w1d[0][:, b0 * 128:(b0 + nb) * 128]
                    .rearrange("p (n m) -> p n m", m=128))

            def dma_w2_l(l):
                b0, nb = p0.w2_range_of_l[l]
                tg = l if G1 == 1 else l % 2
                w2t[l] = sb.tile([128, nb, 128], LDT[0], name=f"w2L0_{l}",
                                 tag=f"w2L0_{tg}")
                nc.sync.dma_start(
                    out=w2t[l][:, :, :],
                    in_=w2d[0][:, b0 * 128:(b0 + nb) * 128]
                    .rearrange("p (n m) -> p n m", m=128))

            h2 = sb.tile([128, p0.lout.ntiles, BPC], LDT[1], name="h_1",
                         tag="hB")
            if plans[1].dr:
                zero_pads(h2, p0.lout, range(p0.lout.ntiles), BPC)

            def l1_g1_chain(l, mt, mtloc, midl, cs):
                V = int(p0.lmid.valid[mt])
                its = p0.g1_chains[mt]
                b0_l = p0.w1_range_of_l[l][0]
                pm = ps.tile([128, CW], F32, name="pm_0", tag="pmid", bufs=3)
                if p0.dr:
                    j = 0
                    first = True
                    while j < len(its):
                        t0 = its[j]
                        k = t0 // KT
                        b = p0.w1_block_of[(mt, t0)] - b0_l
                        lhsT = w1t[l][:, b:b + 2, 0:V]
                        rhs = hx[k][cs][:, (t0 % KT):(t0 % KT) + 2, :]
                        nc.tensor.matmul(pm[0:V, :], lhsT, rhs,
                                         start=first, stop=(j + 2 >= len(its)),
                                         perf_mode=mybir.MatmulPerfMode.DoubleRow)
                        first = False
                        j += 2
                else:
                    for j, it in enumerate(its):
                        ln = int(p0.lin.valid[it])
                        k = it // KT
                        b = p0.w1_block_of[(mt, it)] - b0_l
                        lhsT = w1t[l][0:ln, b, 0:V]
                        rhs = hx[k][cs][0:ln, it % KT, :]
                        nc.tensor.matmul(pm[0:V, :], lhsT, rhs,
                                         start=(j == 0),
                                         stop=(j == len(its) - 1))
                evict(midl[0:V, mtloc, :], pm[0:V, :], relu=False)

            def l1_g2_chain(l, ot, mid_of, cs):
                V = int(p0.lout.valid[ot])
                mts = p0.g2_chains[ot]
                b0_l = p0.w2_range_of_l[l][0]
                po = ps.tile([128, CW], F32, name="po_0", tag="pout", bufs=3)
                if p0.dr:
                    for j in range(0, len(mts), 2):
                        mt = mts[j]
                        b = p0.w2_block_of[(ot, mt)] - b0_l
                        midl, loc = mid_of[mt]
                        lhsT = w2t[l][:, b:b + 2, 0:V]
                        rhs = midl[:, loc:loc + 2, :]
                        nc.tensor.matmul(po[0:V, :], lhsT, rhs,
                                         start=(j == 0),
                                         stop=(j + 2 >= len(mts)),
                                         perf_mode=mybir.MatmulPerfMode.DoubleRow)
                else:
                    for j, mt in enumerate(mts):
                        ln = int(p0.lmid.valid[mt])
                        b = p0.w2_block_of[(ot, mt)] - b0_l
                        midl, loc = mid_of[mt]
                        lhsT = w2t[l][0:ln, b, 0:V]
                        rhs = midl[0:ln, loc, :]
                        nc.tensor.matmul(po[0:V, :], lhsT, rhs,
                                         start=(j == 0),
                                         stop=(j == len(mts) - 1))
                evict(h2[0:V, ot, cs * CW:(cs + 1) * CW], po[0:V, :],
                      relu=True)

            # head DMA interleave + L1 emission (cs-outer, per-l lookahead)
            for g in range(G1):
                ls = list(range(4)) if G1 == 1 else [2 * g, 2 * g + 1]
                if g == 0:
                    dma_x(0, 0)
                    dma_w1_l(ls[0])
                    dma_x(1, 0)
                    dma_x(2, 0)
                    dma_x(3, 0)
                    dma_w1_l(ls[1])
                    dma_w2_l(ls[0])
                    dma_x(0, 1)
                    dma_x(1, 1)
                    dma_w2_l(ls[1])
                    if G1 == 1:
                        dma_w1_l(2)
                    dma_x(2, 1)
                    dma_x(3, 1)
                    if G1 == 1:
                        dma_w1_l(3)
                        dma_w2_l(2)
                        dma_w2_l(3)
                else:
                    dma_w1_l(2)
                    dma_w1_l(3)
                    dma_w2_l(2)
                    dma_w2_l(3)

                for cs in range(CS):
                    mid_of = {}
                    pend = None
                    for l in ls:
                        mts_l = p0.mid_tiles_of_l[l]
                        midl = sb.tile([128, len(mts_l), CW], LDT[0],
                                       name=f"mid_0_{l}_{cs}", tag="midb",
                                       bufs=3)
                        if p0.dr:
                            zero_pads(midl, p0.lmid, mts_l, CW)
                        for loc, mt in enumerate(mts_l):
                            mid_of[mt] = (midl, loc)
                            l1_g1_chain(l, mt, loc, midl, cs)
                        if pend is not None:
                            for ot in p0.out_tiles_of_l[pend]:
                                l1_g2_chain(pend, ot, mid_of, cs)
                        pend = l
                    for ot in p0.out_tiles_of_l[pend]:
                        l1_g2_chain(pend, ot, mid_of, cs)

            # ====================== layers 2..NMONARCH ======================
            hin = h2
            for li in range(1, NMONARCH):
                pl = plans[li]
                w1sb = sb.tile([128, len(pl.w1_blocks), 128], LDT[li],
                               name=f"w1sb_{li}", tag="w1", bufs=2)
                nc.sync.dma_start(
                    out=w1sb[:, :, :],
                    in_=w1d[li][:, :].rearrange("p (n m) -> p n m", m=128))
                w2sb = sb.tile([128, len(pl.w2_blocks), 128], LDT[li],
                               name=f"w2sb_{li}", tag="w2", bufs=2)
                nc.sync.dma_start(
                    out=w2sb[:, :, :],
                    in_=w2d[li][:, :].rearrange("p (n m) -> p n m", m=128))

                hnext = sb.tile([128, pl.lout.ntiles, BPC], LDT[li + 1],
                                name=f"h_{li + 1}",
                                tag="hA" if li % 2 == 1 else "hB")
                if plans[li + 1].dr:
                    zero_pads(hnext, pl.lout, range(pl.lout.ntiles), BPC)

                def g1_tile(mt, mtloc, midl, cs):
                    V = int(pl.lmid.valid[mt])
                    its = pl.g1_chains[mt]
                    c0 = cs * CW
                    pm = ps.tile([128, CW], F32, name=f"pm_{li}", tag="pmid",
                                 bufs=3)
                    if pl.dr:
                        for j in range(0, len(its), 2):
                            it = its[j]
                            b = pl.w1_block_of[(mt, it)]
                            lhsT = w1sb[:, b:b + 2, 0:V]
                            rhs = hin[:, it:it + 2, c0:c0 + CW]
                            nc.tensor.matmul(
                                pm[0:V, :], lhsT, rhs,
                                start=(j == 0), stop=(j + 2 >= len(its)),
                                perf_mode=mybir.MatmulPerfMode.DoubleRow)
                    else:
                        for j, it in enumerate(its):
                            ln = int(pl.lin.valid[it])
                            b = pl.w1_block_of[(mt, it)]
                            lhsT = w1sb[0:ln, b, 0:V]
                            rhs = hin[0:ln, it, c0:c0 + CW]
                            nc.tensor.matmul(pm[0:V, :], lhsT, rhs,
                                             start=(j == 0),
                                             stop=(j == len(its) - 1))
                    evict(midl[0:V, mtloc, c0:c0 + CW], pm[0:V, :],
                          relu=False)

                def g2_tile(ot, mid_of, cs):
                    V = int(pl.lout.valid[ot])
                    mts = pl.g2_chains[ot]
                    c0 = cs * CW
                    po = ps.tile([128, CW], F32, name=f"po_{li}", tag="pout",
                                 bufs=3)
                    if pl.dr:
                        for j in range(0, len(mts), 2):
                            mt = mts[j]
                            b = pl.w2_block_of[(ot, mt)]
                            midl, loc = mid_of[mt]
                            lhsT = w2sb[:, b:b + 2, 0:V]
                            rhs = midl[:, loc:loc + 2, c0:c0 + CW]
                            nc.tensor.matmul(
                                po[0:V, :], lhsT, rhs,
                                start=(j == 0), stop=(j + 2 >= len(mts)),
                                perf_mode=mybir.MatmulPerfMode.DoubleRow)
                    else:
                        for j, mt in enumerate(mts):
                            ln = int(pl.lmid.valid[mt])
                            b = pl.w2_block_of[(ot, mt)]
                            midl, loc = mid_of[mt]
                            lhsT = w2sb[0:ln, b, 0:V]
                            rhs = midl[0:ln, loc, c0:c0 + CW]
                            nc.tensor.matmul(po[0:V, :], lhsT, rhs,
                                             start=(j == 0),
                                             stop=(j == len(mts) - 1))
                    evict(hnext[0:V, ot, c0:c0 + CW], po[0:V, :], relu=True)

                mid_of = {}
                pend = None
                for l in range(4):
                    mts_l = pl.mid_tiles_of_l[l]
                    midl = sb.tile([128, len(mts_l), BPC], LDT[li],
                                   name=f"mid_{li}_{l}", tag="midb2", bufs=2)
                    if pl.dr:
                        zero_pads(midl, pl.lmid, mts_l, BPC)
                    for loc, mt in enumerate(mts_l):
                        mid_of[mt] = (midl, loc)
                        for cs in range(CS):
                            g1_tile(mt, loc, midl, cs)
                    if pend is not None:
                        for ot in pl.out_tiles_of_l[pend]:
                            for cs in range(CS):
                                g2_tile(ot, mid_of, cs)
                    pend = l
                for ot in pl.out_tiles_of_l[pend]:
                    for cs in range(CS):
                        g2_tile(ot, mid_of, cs)
                hin = hnext

            # ======================= fused layers 4-6 =======================
            for li in range(NMONARCH, NLAYERS - 1):
                pl = plans[li]
                wsb = sb.tile([128, len(pl.blocks), 128], F16,
                              name=f"wsb_{li}", tag="w1", bufs=2)
                nc.sync.dma_start(
                    out=wsb[:, :, :],
                    in_=wfd[li][:, :].rearrange("p (n m) -> p n m", m=128))
                hnext = sb.tile([128, pl.lout.ntiles, BPC], F16,
                                name=f"h_{li + 1}",
                                tag="hA" if li % 2 == 1 else "hB")
                for cs in range(CS):
                    c0 = cs * CW
                    for ot in range(pl.lout.ntiles):
                        V = int(pl.lout.valid[ot])
                        its = pl.chains[ot]
                        po = ps.tile([128, CW], F32, name=f"po_{li}",
                                     tag="pout", bufs=3)
                        for j, it in enumerate(its):
                            ln = int(pl.lin.valid[it])
                            b = pl.block_of[(ot, it)]
                            nc.tensor.matmul(po[0:V, :], wsb[0:ln, b, 0:V],
                                             hin[0:ln, it, c0:c0 + CW],
                                             start=(j == 0),
                                             stop=(j == len(its) - 1))
                        evict(hnext[0:V, ot, c0:c0 + CW], po[0:V, :],
                              relu=True)
                hin = hnext

            # ============== layer 7 (batch-major) + log_softmax ==============
            pf_pl = plans[NLAYERS - 1]
            VF = int(pf_pl.lin.valid[0])
            w7sb = sb.tile([128, NOUT], F16, name="w7sb", tag="w7")
            nc.sync.dma_start(out=w7sb[:, :], in_=wfd[NLAYERS - 1][:, :])

            nchunk = BPC // 128
            esum = sb.tile([128, nchunk], F32, name="esum", tag="esum")
            esb = sb.tile([128, nchunk, NOUT], F32, name="esb", tag="esb")
            tlog = sb.tile([128, nchunk, NOUT], F32, name="tlog", tag="tlog")
            osb = sb.tile([128, nchunk, NOUT], F32, name="osb", tag="osb")
            lse = sb.tile([128, nchunk], F32, name="lse", tag="lse")
            for bc in range(nchunk):
                pf = ps.tile([128, NOUT], F32, name="pfin", tag="pfin",
                             bufs=2)
                nc.tensor.matmul(pf[:, :],
                                 hin[0:VF, 0, bc * 128:(bc + 1) * 128],
                                 w7sb[0:VF, :], start=True, stop=True)
                nc.scalar.activation(esb[:, bc, :], pf,
                                     mybir.ActivationFunctionType.Exp)
                nc.vector.tensor_copy(tlog[:, bc, :], pf)
            nc.vector.reduce_sum(esum, esb, axis=mybir.AxisListType.X)
            nc.scalar.activation(lse, esum, mybir.ActivationFunctionType.Ln)
            for bc in range(nchunk):
                nc.vector.tensor_scalar_sub(osb[:, bc, :], tlog[:, bc, :],
                                            lse[:, bc:bc + 1])
                nc.sync.dma_start(out=y[bc * 128:(bc + 1) * 128, :],
                                  in_=osb[:, bc, :])
    nc.finalize()
    return nc


# --------------------------------------------------- numpy model of the schedule
def numpy_forward(plans, weights, xT):
    """Mirror the device schedule (incl. quantization) for validation."""
    import ml_dtypes

    def npdt(dt):
        return mybir.dt.np(dt)

    B = xT.shape[1]
    h = np.zeros((plans[0].lin.ntiles * 128, B), np.float32)
    h[:xT.shape[0]] = xT.astype(npdt(LDT[0])).astype(np.float32)
    for pl in plans[:NLAYERS - 1]:
        li = pl.li
        if not pl.fused:
            W1m, W2m = weights[li]
            W1m = W1m.astype(np.float32)
            W2m = W2m.astype(np.float32)
            mid = np.zeros((pl.lmid.ntiles * 128, B), np.float32)
            for mt, its in pl.g1_chains.items():
                V = int(pl.lmid.valid[mt])
                acc = np.zeros((V, B), np.float32)
                for it in its:
                    ln = int(pl.lin.valid[it])
                    b = pl.w1_block_of[(mt, it)]
                    acc += W1m[0:ln, b * 128:b * 128 + V].T @ \
                        h[it * 128: it * 128 + ln]
                mid[mt * 128: mt * 128 + V] = acc
            mid = mid.astype(npdt(LDT[li])).astype(np.float32)
            out = np.zeros((pl.lout.ntiles * 128, B), np.float32)
            for ot, mts in pl.g2_chains.items():
                V = int(pl.lout.valid[ot])
                acc = np.zeros((V, B), np.float32)
                for mt in mts:
                    ln = int(pl.lmid.valid[mt])
                    b = pl.w2_block_of[(ot, mt)]
                    acc += W2m[0:ln, b * 128:b * 128 + V].T @ \
                        mid[mt * 128: mt * 128 + ln]
                out[ot * 128: ot * 128 + V] = acc
        else:
            Em = weights[li].astype(np.float32)
            out = np.zeros((pl.lout.ntiles * 128, B), np.float32)
            for ot, its in pl.chains.items():
                V = int(pl.lout.valid[ot])
                acc = np.zeros((V, B), np.float32)
                for it in its:
                    ln = int(pl.lin.valid[it])
                    b = pl.block_of[(ot, it)]
                    acc += Em[0:ln, b * 128:b * 128 + V].T @ \
                        h[it * 128: it * 128 + ln]
                out[ot * 128: ot * 128 + V] = acc
        out = np.maximum(out, 0.0)
        h = out.astype(npdt(LDT[li + 1])).astype(np.float32)
    E7 = weights[NLAYERS - 1].astype(np.float32)
    VF = int(plans[NLAYERS - 1].lin.valid[0])
    logits = (E7[0:VF, :].T @ h[0:VF]).T
    S = np.exp(logits).sum(axis=1, keepdims=True)
    return logits - np.log(S)


# ------------------------------------------------------------------ entry point
def _prep_inputs(inputs, plans):
    x = np.ascontiguousarray(np.asarray(inputs["x"], dtype=np.float32))
    shared = {}
    for i, pl in enumerate(plans):
        w1 = np.asarray(inputs[f"w1_{i + 1}"], dtype=np.float32)
        w2 = np.asarray(inputs[f"w2_{i + 1}"], dtype=np.float32)
        if not pl.fused:
            np_dt = mybir.dt.np(LDT[i])
            W1m, W2m = pl.build_weights(w1, w2)
            shared[f"w1c_{i}"] = np.ascontiguousarray(W1m.astype(np_dt))
            shared[f"w2c_{i}"] = np.ascontiguousarray(W2m.astype(np_dt))
        else:
            Em = pl.build_weights(w1, w2)
            shared[f"wc_{i}"] = np.ascontiguousarray(
                Em.astype(mybir.dt.np(F16)))
    np_x = mybir.dt.np(LDT[0])
    in_maps = []
    for c in range(NCORES):
        m = dict(shared)
        xc = x[c * BPC:(c + 1) * BPC].T.astype(np_x)
        m["xT"] = np.ascontiguousarray(
            xc.reshape(plans[0].lin.ntiles, 128, BPC))
        in_maps.append(m)
    return in_maps


def _run(inputs, trace=False, **spmd_kwargs):
    plans = build_plans()
    in_maps = _prep_inputs(inputs, plans)
    nc = build_program(plans)
    res = run_bass_kernel_spmd(nc, in_maps, core_ids=list(range(NCORES)),
                               trace=trace, **spmd_kwargs)
    out = np.concatenate([r["y"] for r in res.results], axis=0)
    return out.astype(np.float32), res


def kernel(**inputs):
    out, _ = _run(inputs, trace=False)
    return out
